# revision 7
# baseline (speedup 1.0000x reference)
"""Trainium2 Bass kernel for nn_NoiseFilter.

Math (negacyclic-transform direct complex product, validated to 2e-14 in f64):
per frame (noise u[256], amp[65]):
    x  = 2u - 1                      (folded into the host-side fp16 cast)
    X' = x @ A        # [512] = (Re | Im) of 256 odd-frequency (negacyclic) bins
    H' = amp @ B      # [512]
    p  = XR*HR - XI*HI               # Re(X'H')  [256]
    q  = XR*HI + XI*HR               # Im(X'H')  [256]
    out = [p|q] @ E   # [256]  negacyclic inverse == linear conv (support 511)

The odd-frequency (negacyclic) DFT has no degenerate real bins: exactly 256
generic complex bins = 512 real slots, so the complex product is 4 bulk
multiplies + 2 bulk add/subs with no special-cased slots.

On-chip dataflow per 512-frame block (inputs host-pre-transposed to
[coeff, frame] layout and pre-cast to fp16, so no on-chip transposes):
    xt   [128,2,512]  <- DMA                 (time-major noise)
    at   [65,512]     <- DMA                 (amp)
    H'   = B-chunks @ at    -> PSUM -> Pool copy  -> h_sb fp16
    X'   = A-chunks @ xt    -> PSUM -> ACT  copy  -> x_sb fp16
    t1..t4, p, q  on DVE (fp16 SBUF, 2x mode)     -> m_sb fp16
    out  = sum_c E[c]^T @ m[c]  -> PSUM -> Pool copy -> DMA (fp16, [t, frame])
Host transposes the [256, frames] fp16 result back and casts fp32.

Engine budget per block: PE 4267 ns (bound), DVE ~3.6 us, ACT ~2.1 us,
Pool ~2.3 us, DMA ~1.5 us.  Data parallel over 8 cores: 8192 frames/core.
"""

import os

import numpy as np

os.environ.setdefault("MYCRO_LOCAL_CACHE", "1")

HOP = 256
NB = 65
B_DIM = 16
F_DIM = 4096
NCORES = 8
FRAMES = B_DIM * F_DIM
FR_PER_CORE = FRAMES // NCORES  # 8192
BLK = 512                        # frames per block


# ---------------------------------------------------------------- matrices
def _build_matrices():
    FS = 128
    N = 512
    t = np.arange(HOP)
    s = np.arange(256)
    # negacyclic (odd-frequency) analysis: X'[s] = sum_t x[t] e^{-2pi i t (s+1/2)/N}
    W = np.exp(-2j * np.pi * np.outer(t, s + 0.5) / N)   # [256, 256]
    A = np.concatenate([W.real, W.imag], axis=1)          # [256, 512]

    eye = np.eye(NB)
    ir = np.fft.irfft(eye, axis=-1)                       # [65, 128]
    ir = np.roll(ir, FS // 2, axis=-1)
    n = np.arange(FS)
    win = 0.5 * (1.0 - np.cos(2.0 * np.pi * n / FS))
    ir = ir * win
    ir = np.pad(ir, ((0, 0), (0, HOP - FS)))
    M_imp = np.roll(ir, -(FS // 2), axis=-1)              # [65, 256]

    D = M_imp @ W                                         # [65, 256] complex
    Bm = np.concatenate([D.real, D.imag], axis=1)         # [65, 512]

    # inverse: out[n] = (2/N) Re sum_s Y'[s] e^{+2pi i n (s+1/2)/N}
    nn = np.arange(HOP)
    Winv = np.exp(2j * np.pi * np.outer(s + 0.5, nn) / N)  # [256, 256]
    E = np.concatenate([(2.0 / N) * Winv.real,             # multiplies p
                        -(2.0 / N) * Winv.imag], axis=0)   # multiplies q

    return (np.ascontiguousarray(A, np.float16),
            np.ascontiguousarray(Bm, np.float16),
            np.ascontiguousarray(E, np.float16))


# ---------------------------------------------------------------- bass kernel
def _emit_kernel(ctx, tc, xt_d, amp_d, a_cst, b_cst, e_cst, out_d, n_frames):
    import concourse.mybir as mybir

    nc = tc.nc
    f32 = mybir.dt.float32
    f16 = mybir.dt.float16
    Copy = mybir.ActivationFunctionType.Copy
    mult = mybir.AluOpType.mult
    add = mybir.AluOpType.add
    sub = mybir.AluOpType.subtract

    nblk = n_frames // BLK
    PIPE = 2  # inverse trails the forward pass by 2 blocks

    singles = ctx.enter_context(tc.tile_pool(name="singles", bufs=1))
    p_xt = ctx.enter_context(tc.tile_pool(name="p_xt", bufs=3))
    p_at = ctx.enter_context(tc.tile_pool(name="p_at", bufs=3))
    p_h = ctx.enter_context(tc.tile_pool(name="p_h", bufs=3))
    p_x = ctx.enter_context(tc.tile_pool(name="p_x", bufs=3))
    p_t = ctx.enter_context(tc.tile_pool(name="p_t", bufs=2))
    p_m = ctx.enter_context(tc.tile_pool(name="p_m", bufs=PIPE + 2))
    p_o = ctx.enter_context(tc.tile_pool(name="p_o", bufs=3))
    ps_b = ctx.enter_context(tc.tile_pool(name="ps_b", bufs=1, space="PSUM"))
    ps_a = ctx.enter_context(tc.tile_pool(name="ps_a", bufs=2, space="PSUM"))
    ps_o = ctx.enter_context(tc.tile_pool(name="ps_o", bufs=1, space="PSUM"))

    # constants
    a_sb = singles.tile([128, 2, 4, 128], f16)
    nc.sync.dma_start(out=a_sb, in_=a_cst.rearrange(
        "(kt p) (c s) -> p kt c s", p=128, s=128))
    b_sb = singles.tile([NB, 4, 128], f16)
    nc.sync.dma_start(out=b_sb, in_=b_cst.rearrange("k (c s) -> k c s", s=128))
    e_sb = singles.tile([128, 4, 2, 128], f16)
    nc.sync.dma_start(out=e_sb, in_=e_cst.rearrange(
        "(c p) (j t) -> p c j t", p=128, t=128))

    xv = xt_d.rearrange("(kt p) (nb f) -> nb p kt f", p=128, f=BLK)
    av = amp_d.rearrange("k (nb f) -> nb k f", f=BLK)
    ov = out_d.rearrange("(jt p) (nb f) -> nb p jt f", p=128, f=BLK)

    m_ring = {}

    for b in range(nblk + PIPE):
        if b < nblk:
            # ---- loads (already fp16, already coeff-major)
            xt = p_xt.tile([128, 2, BLK], f16, tag="xt")
            nc.sync.dma_start(out=xt, in_=xv[b])
            at = p_at.tile([NB, BLK], f16, tag="at")
            nc.sync.dma_start(out=at, in_=av[b])

            # ---- H' = amp @ B   (4 chunk matmuls, Pool copies to fp16)
            h_sb = p_h.tile([128, 4, BLK], f16, tag="h")
            for half in range(2):
                pb = ps_b.tile([128, 2, BLK], f32, tag="pb")
                for cc in range(2):
                    c = half * 2 + cc
                    nc.tensor.matmul(pb[:, cc, :], b_sb[:, c, :], at,
                                     start=True, stop=True)
                nc.scalar.activation(out=h_sb[:, half * 2:half * 2 + 2, :],
                                     in_=pb, func=Copy)

            # ---- X' = x @ A    (8 matmuls, ACT copies to fp16)
            x_sb = p_x.tile([128, 4, BLK], f16, tag="x")
            for half in range(2):
                pa = ps_a.tile([128, 2, BLK], f32, tag="pa")
                for cc in range(2):
                    c = half * 2 + cc
                    for k in range(2):
                        nc.tensor.matmul(pa[:, cc, :], a_sb[:, k, c, :],
                                         xt[:, k, :],
                                         start=(k == 0), stop=(k == 1))
                nc.scalar.activation(out=x_sb[:, half * 2:half * 2 + 2, :],
                                     in_=pa, func=Copy)

            # ---- complex product (DVE fp16 2x): slots [XR|XI] x [HR|HI]
            t_sb = p_t.tile([128, 8, BLK], f16, tag="t")
            nc.vector.tensor_tensor(out=t_sb[:, 0:2, :], in0=x_sb[:, 0:2, :],
                                    in1=h_sb[:, 0:2, :], op=mult)  # RR
            nc.vector.tensor_tensor(out=t_sb[:, 2:4, :], in0=x_sb[:, 2:4, :],
                                    in1=h_sb[:, 2:4, :], op=mult)  # II
            nc.vector.tensor_tensor(out=t_sb[:, 4:6, :], in0=x_sb[:, 0:2, :],
                                    in1=h_sb[:, 2:4, :], op=mult)  # RI
            nc.vector.tensor_tensor(out=t_sb[:, 6:8, :], in0=x_sb[:, 2:4, :],
                                    in1=h_sb[:, 0:2, :], op=mult)  # IR
            m_sb = p_m.tile([128, 4, BLK], f16, tag="m")
            nc.gpsimd.tensor_tensor(out=m_sb[:, 0:2, :], in0=t_sb[:, 0:2, :],
                                    in1=t_sb[:, 2:4, :], op=sub)   # p
            nc.gpsimd.tensor_tensor(out=m_sb[:, 2:4, :], in0=t_sb[:, 4:6, :],
                                    in1=t_sb[:, 6:8, :], op=add)   # q
            m_ring[b] = m_sb

        if b >= PIPE:
            bb = b - PIPE
            m_sb = m_ring.pop(bb)
            po = ps_o.tile([128, 2, BLK], f32, tag="po")
            for j in range(2):
                for c in range(4):
                    nc.tensor.matmul(po[:, j, :], e_sb[:, c, j, :],
                                     m_sb[:, c, :],
                                     start=(c == 0), stop=(c == 3))
            o_sb = p_o.tile([128, 2, BLK], f16, tag="o")
            nc.vector.tensor_copy(o_sb, po)
            nc.sync.dma_start(out=ov[bb], in_=o_sb)


def build_nc(n_frames=FR_PER_CORE):
    import concourse.bacc as bacc
    import concourse.mybir as mybir
    import concourse.tile as tile

    f16 = mybir.dt.float16
    nc = bacc.Bacc("TRN2", target_bir_lowering=False, debug=False)
    xt_d = nc.dram_tensor("xt", [HOP, n_frames], f16, kind="ExternalInput").ap()
    amp_d = nc.dram_tensor("ampt", [NB, n_frames], f16, kind="ExternalInput").ap()
    a_cst = nc.dram_tensor("a_cst", [HOP, 512], f16, kind="ExternalInput").ap()
    b_cst = nc.dram_tensor("b_cst", [NB, 512], f16, kind="ExternalInput").ap()
    e_cst = nc.dram_tensor("e_cst", [512, HOP], f16, kind="ExternalInput").ap()
    out_d = nc.dram_tensor("out", [HOP, n_frames], f16, kind="ExternalOutput").ap()

    from contextlib import ExitStack

    with tile.TileContext(nc) as tc, ExitStack() as ctx:
        _emit_kernel(ctx, tc, xt_d, amp_d, a_cst, b_cst, e_cst, out_d, n_frames)
    nc.compile()
    return nc


_CACHE = {}


def _get(n_frames=FR_PER_CORE):
    key = n_frames
    if key not in _CACHE:
        _CACHE[key] = (build_nc(n_frames), _build_matrices())
    return _CACHE[key]


def run_sharded(noise_flat, amp_flat, n_frames_per_core, n_cores, trace=False):
    """noise_flat: [n, 256] fp32 u-noise; amp_flat: [n, 65] fp32."""
    from concourse import bass_utils

    nc, (A, Bm, E) = _get(n_frames_per_core)
    x16 = (2.0 * noise_flat - 1.0).astype(np.float16)
    a16 = amp_flat.astype(np.float16)
    in_maps = []
    for i in range(n_cores):
        lo, hi = i * n_frames_per_core, (i + 1) * n_frames_per_core
        in_maps.append({
            "xt": np.ascontiguousarray(x16[lo:hi].T),
            "ampt": np.ascontiguousarray(a16[lo:hi].T),
            "a_cst": A, "b_cst": Bm, "e_cst": E,
        })
    res = bass_utils.run_bass_kernel_spmd(
        nc, in_maps, core_ids=list(range(n_cores)), trace=trace
    )
    out = np.concatenate(
        [res.results[i]["out"].T for i in range(n_cores)], axis=0)
    return out.astype(np.float32), res


def kernel(filter_bank, noise_u):
    fb = np.asarray(filter_bank, np.float32).reshape(-1, NB)
    nu = np.asarray(noise_u, np.float32).reshape(-1, HOP)
    out, _ = run_sharded(nu, fb, FR_PER_CORE, NCORES)
    return out.reshape(B_DIM, F_DIM * HOP, 1).astype(np.float32)


if __name__ == "__main__":
    nc = build_nc(BLK * 2)
    print("built OK")


# revision 10
# speedup vs baseline: 1.0030x; 1.0030x over previous
"""Trainium2 Bass kernel for nn_NoiseFilter.

Math (negacyclic-transform direct complex product, validated to 2e-14 in f64):
per frame (noise u[256], amp[65]):
    x  = 2u - 1                      (folded into the host-side fp16 cast)
    X' = x @ A        # [512] = (Re | Im) of 256 odd-frequency (negacyclic) bins
    H' = amp @ B      # [512]
    p  = XR*HR - XI*HI               # Re(X'H')  [256]
    q  = XR*HI + XI*HR               # Im(X'H')  [256]
    out = [p|q] @ E   # [256]  negacyclic inverse == linear conv (support 511)

The odd-frequency (negacyclic) DFT has no degenerate real bins: exactly 256
generic complex bins = 512 real slots, so the complex product is 4 bulk
multiplies + 2 bulk add/subs with no special-cased slots.

On-chip dataflow per 512-frame block (inputs host-pre-transposed to
[coeff, frame] layout and pre-cast to fp16, so no on-chip transposes):
    xt   [128,2,512]  <- DMA                 (time-major noise)
    at   [65,512]     <- DMA                 (amp)
    H'   = B-chunks @ at    -> PSUM -> Pool copy  -> h_sb fp16
    X'   = A-chunks @ xt    -> PSUM -> ACT  copy  -> x_sb fp16
    t1..t4, p, q  on DVE (fp16 SBUF, 2x mode)     -> m_sb fp16
    out  = sum_c E[c]^T @ m[c]  -> PSUM -> Pool copy -> DMA (fp16, [t, frame])
Host transposes the [256, frames] fp16 result back and casts fp32.

Engine budget per block: PE 4267 ns (bound), DVE ~3.6 us, ACT ~2.1 us,
Pool ~2.3 us, DMA ~1.5 us.  Data parallel over 8 cores: 8192 frames/core.
"""

import os

import numpy as np

os.environ.setdefault("MYCRO_LOCAL_CACHE", "1")

HOP = 256
NB = 65
B_DIM = 16
F_DIM = 4096
NCORES = 8
FRAMES = B_DIM * F_DIM
FR_PER_CORE = FRAMES // NCORES  # 8192
BLK = 512                        # frames per block


# ---------------------------------------------------------------- matrices
def _build_matrices():
    FS = 128
    N = 512
    t = np.arange(HOP)
    s = np.arange(256)
    # negacyclic (odd-frequency) analysis: X'[s] = sum_t x[t] e^{-2pi i t (s+1/2)/N}
    W = np.exp(-2j * np.pi * np.outer(t, s + 0.5) / N)   # [256, 256]
    A = np.concatenate([W.real, W.imag], axis=1)          # [256, 512]

    eye = np.eye(NB)
    ir = np.fft.irfft(eye, axis=-1)                       # [65, 128]
    ir = np.roll(ir, FS // 2, axis=-1)
    n = np.arange(FS)
    win = 0.5 * (1.0 - np.cos(2.0 * np.pi * n / FS))
    ir = ir * win
    ir = np.pad(ir, ((0, 0), (0, HOP - FS)))
    M_imp = np.roll(ir, -(FS // 2), axis=-1)              # [65, 256]

    D = M_imp @ W                                         # [65, 256] complex
    Bm = np.concatenate([D.real, D.imag], axis=1)         # [65, 512]

    # inverse: out[n] = (2/N) Re sum_s Y'[s] e^{+2pi i n (s+1/2)/N}
    nn = np.arange(HOP)
    Winv = np.exp(2j * np.pi * np.outer(s + 0.5, nn) / N)  # [256, 256]
    E = np.concatenate([(2.0 / N) * Winv.real,             # multiplies p
                        -(2.0 / N) * Winv.imag], axis=0)   # multiplies q

    return (np.ascontiguousarray(A, np.float16),
            np.ascontiguousarray(Bm, np.float16),
            np.ascontiguousarray(E, np.float16))


# ---------------------------------------------------------------- bass kernel
def _emit_kernel(ctx, tc, xt_d, amp_d, a_cst, b_cst, e_cst, out_d, n_frames):
    import concourse.mybir as mybir

    nc = tc.nc
    f32 = mybir.dt.float32
    f16 = mybir.dt.float16
    Copy = mybir.ActivationFunctionType.Copy
    mult = mybir.AluOpType.mult
    add = mybir.AluOpType.add
    sub = mybir.AluOpType.subtract

    nblk = n_frames // BLK
    PIPE = 3  # inverse trails the forward pass by 3 blocks

    singles = ctx.enter_context(tc.tile_pool(name="singles", bufs=1))
    p_xt = ctx.enter_context(tc.tile_pool(name="p_xt", bufs=3))
    p_at = ctx.enter_context(tc.tile_pool(name="p_at", bufs=3))
    p_h = ctx.enter_context(tc.tile_pool(name="p_h", bufs=3))
    p_x = ctx.enter_context(tc.tile_pool(name="p_x", bufs=3))
    p_t = ctx.enter_context(tc.tile_pool(name="p_t", bufs=2))
    p_m = ctx.enter_context(tc.tile_pool(name="p_m", bufs=PIPE + 2))
    p_o = ctx.enter_context(tc.tile_pool(name="p_o", bufs=3))
    ps_b = ctx.enter_context(tc.tile_pool(name="ps_b", bufs=1, space="PSUM"))
    ps_a = ctx.enter_context(tc.tile_pool(name="ps_a", bufs=2, space="PSUM"))
    ps_o = ctx.enter_context(tc.tile_pool(name="ps_o", bufs=1, space="PSUM"))

    # constants
    a_sb = singles.tile([128, 2, 4, 128], f16)
    nc.sync.dma_start(out=a_sb, in_=a_cst.rearrange(
        "(kt p) (c s) -> p kt c s", p=128, s=128))
    b_sb = singles.tile([NB, 4, 128], f16)
    nc.sync.dma_start(out=b_sb, in_=b_cst.rearrange("k (c s) -> k c s", s=128))
    e_sb = singles.tile([128, 4, 2, 128], f16)
    nc.sync.dma_start(out=e_sb, in_=e_cst.rearrange(
        "(c p) (j t) -> p c j t", p=128, t=128))

    xv = xt_d.rearrange("(kt p) (nb f) -> nb p kt f", p=128, f=BLK)
    av = amp_d.rearrange("k (nb f) -> nb k f", f=BLK)
    ov = out_d.rearrange("(jt p) (nb f) -> nb p jt f", p=128, f=BLK)

    m_ring = {}

    for b in range(nblk + PIPE):
        if b < nblk:
            # ---- loads (already fp16, already coeff-major)
            xt = p_xt.tile([128, 2, BLK], f16, tag="xt")
            nc.sync.dma_start(out=xt, in_=xv[b])
            at = p_at.tile([NB, BLK], f16, tag="at")
            nc.sync.dma_start(out=at, in_=av[b])

            # ---- H' = amp @ B, X' = x @ A  (PE), ACT copies to fp16
            h_sb = p_h.tile([128, 4, BLK], f16, tag="h")
            x_sb = p_x.tile([128, 4, BLK], f16, tag="x")
            pb = {}
            pa = {}
            for half in range(2):
                pb_t = ps_b.tile([128, 2, BLK], f32, tag="pb")
                pb[half] = pb_t
                for cc in range(2):
                    c = half * 2 + cc
                    nc.tensor.matmul(pb_t[:, cc, :], b_sb[:, c, :], at,
                                     start=True, stop=True)
            for half in range(2):
                pa_t = ps_a.tile([128, 2, BLK], f32, tag="pa")
                pa[half] = pa_t
                for cc in range(2):
                    c = half * 2 + cc
                    for k in range(2):
                        nc.tensor.matmul(pa_t[:, cc, :], a_sb[:, k, c, :],
                                         xt[:, k, :],
                                         start=(k == 0), stop=(k == 1))
            # copy order h0,x0,h1,x1 so the first products start sooner
            nc.scalar.activation(out=h_sb[:, 0:2, :], in_=pb[0], func=Copy)
            nc.scalar.activation(out=x_sb[:, 0:2, :], in_=pa[0], func=Copy)
            nc.scalar.activation(out=h_sb[:, 2:4, :], in_=pb[1], func=Copy)
            nc.scalar.activation(out=x_sb[:, 2:4, :], in_=pa[1], func=Copy)

            # ---- complex product (DVE fp16 2x): slots [XR|XI] x [HR|HI]
            t_sb = p_t.tile([128, 8, BLK], f16, tag="t")
            nc.vector.tensor_tensor(out=t_sb[:, 0:2, :], in0=x_sb[:, 0:2, :],
                                    in1=h_sb[:, 0:2, :], op=mult)  # RR
            nc.vector.tensor_tensor(out=t_sb[:, 4:6, :], in0=x_sb[:, 0:2, :],
                                    in1=h_sb[:, 2:4, :], op=mult)  # RI
            nc.vector.tensor_tensor(out=t_sb[:, 6:8, :], in0=x_sb[:, 2:4, :],
                                    in1=h_sb[:, 0:2, :], op=mult)  # IR
            nc.vector.tensor_tensor(out=t_sb[:, 2:4, :], in0=x_sb[:, 2:4, :],
                                    in1=h_sb[:, 2:4, :], op=mult)  # II
            m_sb = p_m.tile([128, 4, BLK], f16, tag="m")
            nc.gpsimd.tensor_tensor(out=m_sb[:, 2:4, :], in0=t_sb[:, 4:6, :],
                                    in1=t_sb[:, 6:8, :], op=add)   # q
            nc.vector.tensor_tensor(out=m_sb[:, 0:2, :], in0=t_sb[:, 0:2, :],
                                    in1=t_sb[:, 2:4, :], op=sub)   # p
            m_ring[b] = m_sb

        if b >= PIPE:
            bb = b - PIPE
            m_sb = m_ring.pop(bb)
            po = ps_o.tile([128, 2, BLK], f32, tag="po")
            for j in range(2):
                for c in range(4):
                    nc.tensor.matmul(po[:, j, :], e_sb[:, c, j, :],
                                     m_sb[:, c, :],
                                     start=(c == 0), stop=(c == 3))
            o_sb = p_o.tile([128, 2, BLK], f16, tag="o")
            nc.vector.tensor_copy(o_sb, po)
            nc.sync.dma_start(out=ov[bb], in_=o_sb)


def build_nc(n_frames=FR_PER_CORE):
    import concourse.bacc as bacc
    import concourse.mybir as mybir
    import concourse.tile as tile

    f16 = mybir.dt.float16
    f32 = mybir.dt.float32
    nc = bacc.Bacc("TRN2", target_bir_lowering=False, debug=False)
    xt_d = nc.dram_tensor("xt", [HOP, n_frames], f16, kind="ExternalInput").ap()
    amp_d = nc.dram_tensor("ampt", [NB, n_frames], f16, kind="ExternalInput").ap()
    a_cst = nc.dram_tensor("a_cst", [HOP, 512], f16, kind="ExternalInput").ap()
    b_cst = nc.dram_tensor("b_cst", [NB, 512], f16, kind="ExternalInput").ap()
    e_cst = nc.dram_tensor("e_cst", [512, HOP], f16, kind="ExternalInput").ap()
    out_d = nc.dram_tensor("out", [HOP, n_frames], f16, kind="ExternalOutput").ap()

    from contextlib import ExitStack

    with tile.TileContext(nc) as tc, ExitStack() as ctx:
        _emit_kernel(ctx, tc, xt_d, amp_d, a_cst, b_cst, e_cst, out_d, n_frames)
    nc.compile()
    return nc


_CACHE = {}


def _get(n_frames=FR_PER_CORE):
    key = n_frames
    if key not in _CACHE:
        _CACHE[key] = (build_nc(n_frames), _build_matrices())
    return _CACHE[key]


def run_sharded(noise_flat, amp_flat, n_frames_per_core, n_cores, trace=False):
    """noise_flat: [n, 256] fp32 u-noise; amp_flat: [n, 65] fp32."""
    from concourse import bass_utils

    nc, (A, Bm, E) = _get(n_frames_per_core)
    x16 = (2.0 * noise_flat - 1.0).astype(np.float16)
    a16 = amp_flat.astype(np.float16)
    in_maps = []
    for i in range(n_cores):
        lo, hi = i * n_frames_per_core, (i + 1) * n_frames_per_core
        in_maps.append({
            "xt": np.ascontiguousarray(x16[lo:hi].T),
            "ampt": np.ascontiguousarray(a16[lo:hi].T),
            "a_cst": A, "b_cst": Bm, "e_cst": E,
        })
    res = bass_utils.run_bass_kernel_spmd(
        nc, in_maps, core_ids=list(range(n_cores)), trace=trace
    )
    out = np.concatenate(
        [res.results[i]["out"].T for i in range(n_cores)], axis=0)
    return out.astype(np.float32), res


def kernel(filter_bank, noise_u):
    fb = np.asarray(filter_bank, np.float32).reshape(-1, NB)
    nu = np.asarray(noise_u, np.float32).reshape(-1, HOP)
    out, _ = run_sharded(nu, fb, FR_PER_CORE, NCORES)
    return out.reshape(B_DIM, F_DIM * HOP, 1).astype(np.float32)


if __name__ == "__main__":
    nc = build_nc(BLK * 2)
    print("built OK")


# revision 12
# speedup vs baseline: 1.0345x; 1.0314x over previous
"""Trainium2 Bass kernel for nn_NoiseFilter.

Math (negacyclic-transform direct complex product, validated to 2e-14 in f64):
per frame (noise u[256], amp[65]):
    x  = 2u - 1                      (folded into the host-side fp16 cast)
    X' = x @ A        # [512] = (Re | Im) of 256 odd-frequency (negacyclic) bins
    H' = amp @ B      # [512]
    p  = XR*HR - XI*HI               # Re(X'H')  [256]
    q  = XR*HI + XI*HR               # Im(X'H')  [256]
    out = [p|q] @ E   # [256]  negacyclic inverse == linear conv (support 511)

The odd-frequency (negacyclic) DFT has no degenerate real bins: exactly 256
generic complex bins = 512 real slots, so the complex product is 4 bulk
multiplies + 2 bulk add/subs with no special-cased slots.

On-chip dataflow per 512-frame block (inputs host-pre-transposed to
[coeff, frame] layout and pre-cast to fp16, so no on-chip transposes):
    xt   [128,2,512]  <- DMA                 (time-major noise)
    at   [65,512]     <- DMA                 (amp)
    H'   = B-chunks @ at    -> PSUM -> Pool copy  -> h_sb fp16
    X'   = A-chunks @ xt    -> PSUM -> ACT  copy  -> x_sb fp16
    t1..t4, p, q  on DVE (fp16 SBUF, 2x mode)     -> m_sb fp16
    out  = sum_c E[c]^T @ m[c]  -> PSUM -> Pool copy -> DMA (fp16, [t, frame])
Host transposes the [256, frames] fp16 result back and casts fp32.

Engine budget per block: PE 4267 ns (bound), DVE ~3.6 us, ACT ~2.1 us,
Pool ~2.3 us, DMA ~1.5 us.  Data parallel over 8 cores: 8192 frames/core.
"""

import os

import numpy as np

os.environ.setdefault("MYCRO_LOCAL_CACHE", "1")

HOP = 256
NB = 65
B_DIM = 16
F_DIM = 4096
NCORES = 8
FRAMES = B_DIM * F_DIM
FR_PER_CORE = FRAMES // NCORES  # 8192
BLK = 512                        # frames per block


# ---------------------------------------------------------------- matrices
def _build_matrices():
    FS = 128
    N = 512
    t = np.arange(HOP)
    s = np.arange(256)
    # negacyclic (odd-frequency) analysis: X'[s] = sum_t x[t] e^{-2pi i t (s+1/2)/N}
    W = np.exp(-2j * np.pi * np.outer(t, s + 0.5) / N)   # [256, 256]
    A = np.concatenate([W.real, W.imag], axis=1)          # [256, 512]

    eye = np.eye(NB)
    ir = np.fft.irfft(eye, axis=-1)                       # [65, 128]
    ir = np.roll(ir, FS // 2, axis=-1)
    n = np.arange(FS)
    win = 0.5 * (1.0 - np.cos(2.0 * np.pi * n / FS))
    ir = ir * win
    ir = np.pad(ir, ((0, 0), (0, HOP - FS)))
    M_imp = np.roll(ir, -(FS // 2), axis=-1)              # [65, 256]

    D = M_imp @ W                                         # [65, 256] complex
    Bm = np.concatenate([D.real, D.imag], axis=1)         # [65, 512]

    # inverse: out[n] = (2/N) Re sum_s Y'[s] e^{+2pi i n (s+1/2)/N}
    nn = np.arange(HOP)
    Winv = np.exp(2j * np.pi * np.outer(s + 0.5, nn) / N)  # [256, 256]
    E = np.concatenate([(2.0 / N) * Winv.real,             # multiplies p
                        -(2.0 / N) * Winv.imag], axis=0)   # multiplies q

    return (np.ascontiguousarray(A, np.float16),
            np.ascontiguousarray(Bm, np.float16),
            np.ascontiguousarray(E, np.float16))


# ---------------------------------------------------------------- bass kernel
def _emit_kernel(ctx, tc, xt_d, amp_d, a_cst, b_cst, e_cst, out_d, n_frames):
    import concourse.mybir as mybir

    nc = tc.nc
    f32 = mybir.dt.float32
    f16 = mybir.dt.float16
    Copy = mybir.ActivationFunctionType.Copy
    mult = mybir.AluOpType.mult
    add = mybir.AluOpType.add
    sub = mybir.AluOpType.subtract

    nblk = n_frames // BLK
    PIPE = 3  # inverse trails the forward pass by 3 blocks

    singles = ctx.enter_context(tc.tile_pool(name="singles", bufs=1))
    p_xt = ctx.enter_context(tc.tile_pool(name="p_xt", bufs=3))
    p_at = ctx.enter_context(tc.tile_pool(name="p_at", bufs=3))
    p_h = ctx.enter_context(tc.tile_pool(name="p_h", bufs=3))
    p_x = ctx.enter_context(tc.tile_pool(name="p_x", bufs=3))
    p_t = ctx.enter_context(tc.tile_pool(name="p_t", bufs=2))
    p_m = ctx.enter_context(tc.tile_pool(name="p_m", bufs=PIPE + 2))
    p_o = ctx.enter_context(tc.tile_pool(name="p_o", bufs=3))
    ps_b = ctx.enter_context(tc.tile_pool(name="ps_b", bufs=1, space="PSUM"))
    ps_a = ctx.enter_context(tc.tile_pool(name="ps_a", bufs=2, space="PSUM"))
    ps_o = ctx.enter_context(tc.tile_pool(name="ps_o", bufs=1, space="PSUM"))

    # constants
    a_sb = singles.tile([128, 2, 4, 128], f16)
    nc.sync.dma_start(out=a_sb, in_=a_cst.rearrange(
        "(kt p) (c s) -> p kt c s", p=128, s=128))
    b_sb = singles.tile([NB, 4, 128], f16)
    nc.sync.dma_start(out=b_sb, in_=b_cst.rearrange("k (c s) -> k c s", s=128))
    e_sb = singles.tile([128, 4, 2, 128], f16)

    xv = xt_d.rearrange("(kt p) (nb f) -> nb p kt f", p=128, f=BLK)
    av = amp_d.rearrange("k (nb f) -> nb k f", f=BLK)
    ov = out_d.rearrange("(jt p) (nb f) -> nb p jt f", p=128, f=BLK)

    warm = singles.tile([128, BLK], f16)
    nc.vector.memset(warm, 0.0)
    for w in range(5):
        pw = ps_a.tile([128, 2, BLK], f32, tag="pa")
        nc.tensor.matmul(pw[:, 0, :], warm[:, 0:128], warm,
                         start=True, stop=True)

    m_ring = {}

    for b in range(nblk + PIPE):
        if b < nblk:
            # ---- loads (already fp16, already coeff-major)
            xt = p_xt.tile([128, 2, BLK], f16, tag="xt")
            nc.sync.dma_start(out=xt, in_=xv[b])
            at = p_at.tile([NB, BLK], f16, tag="at")
            nc.sync.dma_start(out=at, in_=av[b])
            if b == 0:
                nc.sync.dma_start(out=e_sb, in_=e_cst.rearrange(
                    "(c p) (j t) -> p c j t", p=128, t=128))

            # ---- H' = amp @ B, X' = x @ A  (PE), ACT copies to fp16
            h_sb = p_h.tile([128, 4, BLK], f16, tag="h")
            x_sb = p_x.tile([128, 4, BLK], f16, tag="x")
            pb = {}
            pa = {}

            po = mo = None
            if b >= PIPE:
                mo = m_ring.pop(b - PIPE)
                po = ps_o.tile([128, 2, BLK], f32, tag="po")
            for half in range(2):
                pb_t = ps_b.tile([128, 2, BLK], f32, tag="pb")
                pb[half] = pb_t
                for cc in range(2):
                    c = half * 2 + cc
                    nc.tensor.matmul(pb_t[:, cc, :], b_sb[:, c, :], at,
                                     start=True, stop=True)
                pa_t = ps_a.tile([128, 2, BLK], f32, tag="pa")
                pa[half] = pa_t
                for cc in range(2):
                    c = half * 2 + cc
                    for k in range(2):
                        nc.tensor.matmul(pa_t[:, cc, :], a_sb[:, k, c, :],
                                         xt[:, k, :],
                                         start=(k == 0), stop=(k == 1))
                # copies for this half (ACT), ordered h then x
                nc.scalar.activation(out=h_sb[:, half * 2:half * 2 + 2, :],
                                     in_=pb_t, func=Copy)
                nc.scalar.activation(out=x_sb[:, half * 2:half * 2 + 2, :],
                                     in_=pa_t, func=Copy)
                # interleaved inverse t-tile j of block b-PIPE
                if po is not None:
                    j = half
                    for c in range(4):
                        nc.tensor.matmul(po[:, j, :], e_sb[:, c, j, :],
                                         mo[:, c, :],
                                         start=(c == 0), stop=(c == 3))
            if po is not None:
                o_sb = p_o.tile([128, 2, BLK], f16, tag="o")
                nc.vector.tensor_copy(o_sb, po)
                nc.sync.dma_start(out=ov[b - PIPE], in_=o_sb)

            # ---- complex product (DVE fp16 2x): slots [XR|XI] x [HR|HI]
            t_sb = p_t.tile([128, 8, BLK], f16, tag="t")
            nc.vector.tensor_tensor(out=t_sb[:, 0:2, :], in0=x_sb[:, 0:2, :],
                                    in1=h_sb[:, 0:2, :], op=mult)  # RR
            nc.vector.tensor_tensor(out=t_sb[:, 4:6, :], in0=x_sb[:, 0:2, :],
                                    in1=h_sb[:, 2:4, :], op=mult)  # RI
            nc.vector.tensor_tensor(out=t_sb[:, 6:8, :], in0=x_sb[:, 2:4, :],
                                    in1=h_sb[:, 0:2, :], op=mult)  # IR
            nc.vector.tensor_tensor(out=t_sb[:, 2:4, :], in0=x_sb[:, 2:4, :],
                                    in1=h_sb[:, 2:4, :], op=mult)  # II
            m_sb = p_m.tile([128, 4, BLK], f16, tag="m")
            nc.gpsimd.tensor_tensor(out=m_sb[:, 2:4, :], in0=t_sb[:, 4:6, :],
                                    in1=t_sb[:, 6:8, :], op=add)   # q
            nc.vector.tensor_tensor(out=m_sb[:, 0:2, :], in0=t_sb[:, 0:2, :],
                                    in1=t_sb[:, 2:4, :], op=sub)   # p
            m_ring[b] = m_sb
        else:
            mo = m_ring.pop(b - PIPE)
            po = ps_o.tile([128, 2, BLK], f32, tag="po")
            for j in range(2):
                for c in range(4):
                    nc.tensor.matmul(po[:, j, :], e_sb[:, c, j, :],
                                     mo[:, c, :],
                                     start=(c == 0), stop=(c == 3))
            o_sb = p_o.tile([128, 2, BLK], f16, tag="o")
            nc.vector.tensor_copy(o_sb, po)
            nc.sync.dma_start(out=ov[b - PIPE], in_=o_sb)




def build_nc(n_frames=FR_PER_CORE):
    import concourse.bacc as bacc
    import concourse.mybir as mybir
    import concourse.tile as tile

    f16 = mybir.dt.float16
    f32 = mybir.dt.float32
    nc = bacc.Bacc("TRN2", target_bir_lowering=False, debug=False)
    xt_d = nc.dram_tensor("xt", [HOP, n_frames], f16, kind="ExternalInput").ap()
    amp_d = nc.dram_tensor("ampt", [NB, n_frames], f16, kind="ExternalInput").ap()
    a_cst = nc.dram_tensor("a_cst", [HOP, 512], f16, kind="ExternalInput").ap()
    b_cst = nc.dram_tensor("b_cst", [NB, 512], f16, kind="ExternalInput").ap()
    e_cst = nc.dram_tensor("e_cst", [512, HOP], f16, kind="ExternalInput").ap()
    out_d = nc.dram_tensor("out", [HOP, n_frames], f16, kind="ExternalOutput").ap()

    from contextlib import ExitStack

    with tile.TileContext(nc) as tc, ExitStack() as ctx:
        _emit_kernel(ctx, tc, xt_d, amp_d, a_cst, b_cst, e_cst, out_d, n_frames)
    nc.compile()
    return nc


_CACHE = {}


def _get(n_frames=FR_PER_CORE):
    key = n_frames
    if key not in _CACHE:
        _CACHE[key] = (build_nc(n_frames), _build_matrices())
    return _CACHE[key]


def run_sharded(noise_flat, amp_flat, n_frames_per_core, n_cores, trace=False):
    """noise_flat: [n, 256] fp32 u-noise; amp_flat: [n, 65] fp32."""
    from concourse import bass_utils

    nc, (A, Bm, E) = _get(n_frames_per_core)
    x16 = (2.0 * noise_flat - 1.0).astype(np.float16)
    a16 = amp_flat.astype(np.float16)
    in_maps = []
    for i in range(n_cores):
        lo, hi = i * n_frames_per_core, (i + 1) * n_frames_per_core
        in_maps.append({
            "xt": np.ascontiguousarray(x16[lo:hi].T),
            "ampt": np.ascontiguousarray(a16[lo:hi].T),
            "a_cst": A, "b_cst": Bm, "e_cst": E,
        })
    res = bass_utils.run_bass_kernel_spmd(
        nc, in_maps, core_ids=list(range(n_cores)), trace=trace
    )
    out = np.concatenate(
        [res.results[i]["out"].T for i in range(n_cores)], axis=0)
    return out.astype(np.float32), res


def kernel(filter_bank, noise_u):
    fb = np.asarray(filter_bank, np.float32).reshape(-1, NB)
    nu = np.asarray(noise_u, np.float32).reshape(-1, HOP)
    out, _ = run_sharded(nu, fb, FR_PER_CORE, NCORES)
    return out.reshape(B_DIM, F_DIM * HOP, 1).astype(np.float32)


if __name__ == "__main__":
    nc = build_nc(BLK * 2)
    print("built OK")


# revision 18
# speedup vs baseline: 1.1258x; 1.0882x over previous
"""Trainium2 Bass kernel for nn_NoiseFilter.

Math (negacyclic-transform direct complex product, validated to 2e-14 in f64):
per frame (noise u[256], amp[65]):
    x  = 2u - 1                      (folded into the host-side fp16 cast)
    X' = x @ A        # [512] = (Re | Im) of 256 odd-frequency (negacyclic) bins
    H' = amp @ B      # [512]
    p  = XR*HR - XI*HI               # Re(X'H')  [256]
    q  = XR*HI + XI*HR               # Im(X'H')  [256]
    out = [p|q] @ E   # [256]  negacyclic inverse == linear conv (support 511)

The odd-frequency (negacyclic) DFT has no degenerate real bins: exactly 256
generic complex bins = 512 real slots, so the complex product is 4 bulk
multiplies + 2 bulk add/subs with no special-cased slots.

On-chip dataflow per 512-frame block (inputs host-pre-transposed to
[coeff, frame] layout and pre-cast to fp16, so no on-chip transposes):
    xt   [128,2,512]  <- DMA                 (time-major noise)
    at   [65,512]     <- DMA                 (amp)
    H'   = B-chunks @ at    -> PSUM -> ACT copy   -> h_sb fp16
    X'   = A-chunks @ xt    -> PSUM -> ACT  copy  -> x_sb fp16
    t1..t4, p, q  on DVE (fp16 SBUF, 2x mode)     -> m_sb fp16
    out  = sum_c E[c]^T @ m[c]  -> PSUM -> DVE copy -> DMA (fp16, [t, frame])
Host transposes the [256, frames] fp16 result back and casts fp32.

Steady-state engine budget per 512-frame block: PE 4443 ns (bound, 20
matmuls x 512 rows), ACT 4152 (4 PSUM->SBUF fp16 spectrum copies), DVE
~4290 (4 products @2x + p-combine + out copies), Pool 2127 (q-combine),
DMA ~1.5 us.  The inverse of block n-2 is interleaved into block n's
forward matmuls; warmup matmuls cover the p-state ramp + DMA startup.
Data parallel over 8 cores: 8192 frames/core.
"""

import os

import numpy as np

os.environ.setdefault("MYCRO_LOCAL_CACHE", "1")

HOP = 256
NB = 65
B_DIM = 16
F_DIM = 4096
NCORES = 8
FRAMES = B_DIM * F_DIM
FR_PER_CORE = FRAMES // NCORES  # 8192
BLK = 512                        # frames per block


# ---------------------------------------------------------------- matrices
def _build_matrices():
    FS = 128
    N = 512
    t = np.arange(HOP)
    s = np.arange(256)
    # negacyclic (odd-frequency) analysis: X'[s] = sum_t x[t] e^{-2pi i t (s+1/2)/N}
    W = np.exp(-2j * np.pi * np.outer(t, s + 0.5) / N)   # [256, 256]
    A = np.concatenate([W.real, W.imag], axis=1)          # [256, 512]

    eye = np.eye(NB)
    ir = np.fft.irfft(eye, axis=-1)                       # [65, 128]
    ir = np.roll(ir, FS // 2, axis=-1)
    n = np.arange(FS)
    win = 0.5 * (1.0 - np.cos(2.0 * np.pi * n / FS))
    ir = ir * win
    ir = np.pad(ir, ((0, 0), (0, HOP - FS)))
    M_imp = np.roll(ir, -(FS // 2), axis=-1)              # [65, 256]

    D = M_imp @ W                                         # [65, 256] complex
    Bm = np.concatenate([D.real, D.imag], axis=1)         # [65, 512]

    # inverse: out[n] = (2/N) Re sum_s Y'[s] e^{+2pi i n (s+1/2)/N}
    nn = np.arange(HOP)
    Winv = np.exp(2j * np.pi * np.outer(s + 0.5, nn) / N)  # [256, 256]
    E = np.concatenate([(2.0 / N) * Winv.real,             # multiplies p
                        -(2.0 / N) * Winv.imag], axis=0)   # multiplies q

    return (np.ascontiguousarray(A, np.float16),
            np.ascontiguousarray(Bm, np.float16),
            np.ascontiguousarray(E, np.float16))


# ---------------------------------------------------------------- bass kernel
def _emit_kernel(ctx, tc, xt_d, amp_d, a_cst, b_cst, e_cst, out_d, n_frames):
    import concourse.mybir as mybir

    nc = tc.nc
    f32 = mybir.dt.float32
    f16 = mybir.dt.float16
    Copy = mybir.ActivationFunctionType.Copy
    mult = mybir.AluOpType.mult
    add = mybir.AluOpType.add
    sub = mybir.AluOpType.subtract

    assert n_frames % BLK == 0
    nfull = n_frames // BLK
    sizes = [BLK] * nfull
    starts = [sum(sizes[:i]) for i in range(len(sizes))]
    nblk = len(sizes)
    PIPE = 2  # inverse trails the forward pass by 2 blocks

    singles = ctx.enter_context(tc.tile_pool(name="singles", bufs=1))
    p_xt = ctx.enter_context(tc.tile_pool(name="p_xt", bufs=3))
    p_at = ctx.enter_context(tc.tile_pool(name="p_at", bufs=3))
    p_h = ctx.enter_context(tc.tile_pool(name="p_h", bufs=3))
    p_x = ctx.enter_context(tc.tile_pool(name="p_x", bufs=3))
    p_t = ctx.enter_context(tc.tile_pool(name="p_t", bufs=2))
    p_m = ctx.enter_context(tc.tile_pool(name="p_m", bufs=PIPE + 2))
    p_o = ctx.enter_context(tc.tile_pool(name="p_o", bufs=3))
    ps_b = ctx.enter_context(tc.tile_pool(name="ps_b", bufs=1, space="PSUM"))
    ps_a = ctx.enter_context(tc.tile_pool(name="ps_a", bufs=2, space="PSUM"))
    ps_o = ctx.enter_context(tc.tile_pool(name="ps_o", bufs=1, space="PSUM"))

    # constants (big ones via SWDGE so the HWDGE queue serves block-0 inputs)
    b_sb = singles.tile([NB, 4, 128], f16)
    nc.sync.dma_start(out=b_sb, in_=b_cst.rearrange("k (c s) -> k c s", s=128))
    a_sb = singles.tile([128, 2, 4, 128], f16)
    nc.gpsimd.dma_start(out=a_sb, in_=a_cst.rearrange(
        "(kt p) (c s) -> p kt c s", p=128, s=128))
    e_sb = singles.tile([128, 4, 2, 128], f16)

    xv = xt_d.rearrange("(kt p) F -> p kt F", p=128)
    av = amp_d
    ov = out_d.rearrange("(jt p) F -> p jt F", p=128)

    # PE warmup: keep PE busy through DMA startup and finish the p-state
    # ramp before real work arrives.
    warm = singles.tile([128, BLK], f16)
    nc.gpsimd.memset(warm, 0.0)
    for w in range(5):
        pw = ps_a.tile([128, 2, BLK], f32, tag="pa")
        nc.tensor.matmul(pw[:, 0, :], warm[:, 0:128], warm,
                         start=True, stop=True)

    m_ring = {}

    for b in range(nblk + PIPE):
        if b < nblk:
            sz = sizes[b]
            lo = starts[b]
            # ---- loads (already fp16, already coeff-major)
            at = p_at.tile([NB, BLK], f16, tag="at")
            nc.sync.dma_start(out=at[:, :sz], in_=av[:, lo:lo + sz])
            xt = p_xt.tile([128, 2, BLK], f16, tag="xt")
            nc.sync.dma_start(out=xt[:, :, :sz], in_=xv[:, :, lo:lo + sz])
            if b == 0:
                nc.gpsimd.dma_start(out=e_sb, in_=e_cst.rearrange(
                    "(c p) (j t) -> p c j t", p=128, t=128))

            # ---- H' = amp @ B, X' = x @ A  (PE), ACT copies to fp16
            h_sb = p_h.tile([128, 4, BLK], f16, tag="h")
            x_sb = p_x.tile([128, 4, BLK], f16, tag="x")
            pb = {}
            pa = {}
            po = mo = None
            o_sb = None
            osz = olo = 0
            if b >= PIPE:
                osz = sizes[b - PIPE]
                olo = starts[b - PIPE]
                mo = m_ring.pop(b - PIPE)
                po_0 = ps_o.tile([128, BLK], f32, tag="po0")
                po_1 = ps_o.tile([128, BLK], f32, tag="po1")
                po = [po_0, po_1]
                o_sb = p_o.tile([128, 2, BLK], f16, tag="o")

            def emit_inv(j, _po=po, _mo=mo, _osz=osz, _olo=olo, _o=o_sb, _b=b):
                for c in range(4):
                    nc.tensor.matmul(_po[j][:, :_osz], e_sb[:, c, j, :],
                                     _mo[:, c, :_osz],
                                     start=(c == 0), stop=(c == 3))
                if _b >= nblk - 1:
                    nc.scalar.activation(out=_o[:, j, :_osz],
                                         in_=_po[j][:, :_osz], func=Copy)
                else:
                    nc.vector.tensor_copy(_o[:, j, :_osz], _po[j][:, :_osz])
                nc.sync.dma_start(out=ov[:, j:j + 1, _olo:_olo + _osz],
                                  in_=_o[:, j:j + 1, :_osz])
            for half in range(2):
                pb_t = ps_b.tile([128, 2, BLK], f32, tag="pb")
                pb[half] = pb_t
                for cc in range(2):
                    c = half * 2 + cc
                    nc.tensor.matmul(pb_t[:, cc, :sz], b_sb[:, c, :],
                                     at[:, :sz], start=True, stop=True)
                pa_t = ps_a.tile([128, 2, BLK], f32, tag="pa")
                pa[half] = pa_t
                for cc in range(2):
                    c = half * 2 + cc
                    for k in range(2):
                        nc.tensor.matmul(pa_t[:, cc, :sz], a_sb[:, k, c, :],
                                         xt[:, k, :sz],
                                         start=(k == 0), stop=(k == 1))
                # copies for this half (ACT), x first (feeds RR)
                nc.scalar.activation(out=x_sb[:, half * 2:half * 2 + 2, :sz],
                                     in_=pa_t[:, :, :sz], func=Copy)
                nc.scalar.activation(out=h_sb[:, half * 2:half * 2 + 2, :sz],
                                     in_=pb_t[:, :, :sz], func=Copy)
                # interleaved inverse t-tile j of block b-PIPE, then its
                # copy + DMA immediately (independent PSUM tile per j)
                if po is not None and b < nblk - 1:
                    emit_inv(half)
                elif b < PIPE:
                    for _w in range(1):
                        pw = ps_a.tile([128, 2, BLK], f32, tag="pa")
                        nc.tensor.matmul(pw[:, 0, :], warm[:, 0:128], warm,
                                         start=True, stop=True)
            if po is not None and b == nblk - 1:
                emit_inv(0)
                emit_inv(1)

            # ---- complex product (DVE fp16 2x): slots [XR|XI] x [HR|HI]
            t_sb = p_t.tile([128, 8, BLK], f16, tag="t")
            m_sb = p_m.tile([128, 4, BLK], f16, tag="m")
            nc.vector.tensor_tensor(out=t_sb[:, 0:2, :sz], in0=x_sb[:, 0:2, :sz],
                                    in1=h_sb[:, 0:2, :sz], op=mult)  # RR
            nc.vector.tensor_tensor(out=t_sb[:, 4:6, :sz], in0=x_sb[:, 0:2, :sz],
                                    in1=h_sb[:, 2:4, :sz], op=mult)  # RI
            nc.vector.tensor_tensor(out=t_sb[:, 6:8, :sz], in0=x_sb[:, 2:4, :sz],
                                    in1=h_sb[:, 0:2, :sz], op=mult)  # IR
            nc.vector.tensor_tensor(out=t_sb[:, 2:4, :sz], in0=x_sb[:, 2:4, :sz],
                                    in1=h_sb[:, 2:4, :sz], op=mult)  # II
            q_eng = nc.vector if b >= nblk - 1 else nc.gpsimd
            q_eng.tensor_tensor(out=m_sb[:, 2:4, :sz],
                                in0=t_sb[:, 4:6, :sz],
                                in1=t_sb[:, 6:8, :sz], op=add)   # q
            nc.vector.tensor_tensor(out=m_sb[:, 0:2, :sz], in0=t_sb[:, 0:2, :sz],
                                    in1=t_sb[:, 2:4, :sz], op=sub)   # p
            m_ring[b] = m_sb

            if b == nblk - 1 and b - 1 >= PIPE - 1 and (b - 1) in m_ring:
                # pull the second-to-last inverse into this block so only one
                # inverse remains in the drain
                osz2 = sizes[b - 1]
                olo2 = starts[b - 1]
                mo2 = m_ring.pop(b - 1)
                o_sb2 = p_o.tile([128, 2, BLK], f16, tag="o")
                for j in range(2):
                    po_j = ps_o.tile([128, BLK], f32, tag=f"po{j}")
                    for c in range(4):
                        nc.tensor.matmul(po_j[:, :osz2], e_sb[:, c, j, :],
                                         mo2[:, c, :osz2],
                                         start=(c == 0), stop=(c == 3))
                    nc.scalar.activation(out=o_sb2[:, j, :osz2],
                                         in_=po_j[:, :osz2], func=Copy)
                    nc.sync.dma_start(out=ov[:, j:j + 1, olo2:olo2 + osz2],
                                      in_=o_sb2[:, j:j + 1, :osz2])
        elif b >= PIPE and (b - PIPE) in m_ring:
            osz = sizes[b - PIPE]
            olo = starts[b - PIPE]
            mo = m_ring.pop(b - PIPE)
            o_sb = p_o.tile([128, 2, BLK], f16, tag="o")
            po_0 = ps_o.tile([128, BLK], f32, tag="po0")
            po_1 = ps_o.tile([128, BLK], f32, tag="po1")
            po_d = [po_0, po_1]
            for j in range(2):
                for c in range(4):
                    nc.tensor.matmul(po_d[j][:, :osz], e_sb[:, c, j, :],
                                     mo[:, c, :osz],
                                     start=(c == 0), stop=(c == 3))
                nc.scalar.activation(out=o_sb[:, j, :osz], in_=po_d[j][:, :osz],
                                     func=Copy)
                nc.sync.dma_start(out=ov[:, j:j + 1, olo:olo + osz],
                                  in_=o_sb[:, j:j + 1, :osz])




def build_nc(n_frames=FR_PER_CORE):
    import concourse.bacc as bacc
    import concourse.mybir as mybir
    import concourse.tile as tile

    f16 = mybir.dt.float16
    f32 = mybir.dt.float32
    nc = bacc.Bacc("TRN2", target_bir_lowering=False, debug=False)
    xt_d = nc.dram_tensor("xt", [HOP, n_frames], f16, kind="ExternalInput").ap()
    amp_d = nc.dram_tensor("ampt", [NB, n_frames], f16, kind="ExternalInput").ap()
    a_cst = nc.dram_tensor("a_cst", [HOP, 512], f16, kind="ExternalInput").ap()
    b_cst = nc.dram_tensor("b_cst", [NB, 512], f16, kind="ExternalInput").ap()
    e_cst = nc.dram_tensor("e_cst", [512, HOP], f16, kind="ExternalInput").ap()
    out_d = nc.dram_tensor("out", [HOP, n_frames], f16, kind="ExternalOutput").ap()

    from contextlib import ExitStack

    with tile.TileContext(nc) as tc, ExitStack() as ctx:
        _emit_kernel(ctx, tc, xt_d, amp_d, a_cst, b_cst, e_cst, out_d, n_frames)
    nc.compile()
    return nc


_CACHE = {}


def _get(n_frames=FR_PER_CORE):
    key = n_frames
    if key not in _CACHE:
        _CACHE[key] = (build_nc(n_frames), _build_matrices())
    return _CACHE[key]


def run_sharded(noise_flat, amp_flat, n_frames_per_core, n_cores, trace=False):
    """noise_flat: [n, 256] fp32 u-noise; amp_flat: [n, 65] fp32."""
    from concourse import bass_utils

    nc, (A, Bm, E) = _get(n_frames_per_core)
    x16 = (2.0 * noise_flat - 1.0).astype(np.float16)
    a16 = amp_flat.astype(np.float16)
    in_maps = []
    for i in range(n_cores):
        lo, hi = i * n_frames_per_core, (i + 1) * n_frames_per_core
        in_maps.append({
            "xt": np.ascontiguousarray(x16[lo:hi].T),
            "ampt": np.ascontiguousarray(a16[lo:hi].T),
            "a_cst": A, "b_cst": Bm, "e_cst": E,
        })
    res = bass_utils.run_bass_kernel_spmd(
        nc, in_maps, core_ids=list(range(n_cores)), trace=trace
    )
    out = np.concatenate(
        [res.results[i]["out"].T for i in range(n_cores)], axis=0)
    return out.astype(np.float32), res


def kernel(filter_bank, noise_u):
    fb = np.asarray(filter_bank, np.float32).reshape(-1, NB)
    nu = np.asarray(noise_u, np.float32).reshape(-1, HOP)
    out, _ = run_sharded(nu, fb, FR_PER_CORE, NCORES)
    return out.reshape(B_DIM, F_DIM * HOP, 1).astype(np.float32)


if __name__ == "__main__":
    nc = build_nc(BLK * 2)
    print("built OK")



# revision 20
# speedup vs baseline: 1.1259x; 1.0001x over previous
"""Trainium2 Bass kernel for nn_NoiseFilter.

Math (negacyclic-transform direct complex product, validated to 2e-14 in f64):
per frame (noise u[256], amp[65]):
    x  = 2u - 1                      (folded into the host-side fp16 cast)
    X' = x @ A        # [512] = (Re | Im) of 256 odd-frequency (negacyclic) bins
    H' = amp @ B      # [512]
    p  = XR*HR - XI*HI               # Re(X'H')  [256]
    q  = XR*HI + XI*HR               # Im(X'H')  [256]
    out = [p|q] @ E   # [256]  negacyclic inverse == linear conv (support 511)

The odd-frequency (negacyclic) DFT has no degenerate real bins: exactly 256
generic complex bins = 512 real slots, so the complex product is 4 bulk
multiplies + 2 bulk add/subs with no special-cased slots.

On-chip dataflow per 512-frame block (inputs host-pre-transposed to
[coeff, frame] layout and pre-cast to fp16, so no on-chip transposes):
    xt   [128,2,512]  <- DMA                 (time-major noise)
    at   [65,512]     <- DMA                 (amp)
    H'   = B-chunks @ at    -> PSUM -> ACT copy   -> h_sb fp16
    X'   = A-chunks @ xt    -> PSUM -> ACT  copy  -> x_sb fp16
    t1..t4, p, q  on DVE (fp16 SBUF, 2x mode)     -> m_sb fp16
    out  = sum_c E[c]^T @ m[c]  -> PSUM -> DVE copy -> DMA (fp16, [t, frame])
Host transposes the [256, frames] fp16 result back and casts fp32.

Steady-state engine budget per 512-frame block: PE 4443 ns (bound, 20
matmuls x 512 rows), ACT 4152 (4 PSUM->SBUF fp16 spectrum copies), DVE
~4290 (4 products @2x + p-combine + out copies), Pool 2127 (q-combine),
DMA ~1.5 us.  The inverse of block n-2 is interleaved into block n's
forward matmuls; warmup matmuls cover the p-state ramp + DMA startup.
Data parallel over 8 cores: 8192 frames/core.
"""

import os

import numpy as np

os.environ.setdefault("MYCRO_LOCAL_CACHE", "1")

HOP = 256
NB = 65
B_DIM = 16
F_DIM = 4096
NCORES = 8
FRAMES = B_DIM * F_DIM
FR_PER_CORE = FRAMES // NCORES  # 8192
BLK = 512                        # frames per block


# ---------------------------------------------------------------- matrices
def _build_matrices():
    FS = 128
    N = 512
    t = np.arange(HOP)
    s = np.arange(256)
    # negacyclic (odd-frequency) analysis: X'[s] = sum_t x[t] e^{-2pi i t (s+1/2)/N}
    W = np.exp(-2j * np.pi * np.outer(t, s + 0.5) / N)   # [256, 256]
    A = np.concatenate([W.real, W.imag], axis=1)          # [256, 512]

    eye = np.eye(NB)
    ir = np.fft.irfft(eye, axis=-1)                       # [65, 128]
    ir = np.roll(ir, FS // 2, axis=-1)
    n = np.arange(FS)
    win = 0.5 * (1.0 - np.cos(2.0 * np.pi * n / FS))
    ir = ir * win
    ir = np.pad(ir, ((0, 0), (0, HOP - FS)))
    M_imp = np.roll(ir, -(FS // 2), axis=-1)              # [65, 256]

    D = M_imp @ W                                         # [65, 256] complex
    Bm = np.concatenate([D.real, D.imag], axis=1)         # [65, 512]

    # inverse: out[n] = (2/N) Re sum_s Y'[s] e^{+2pi i n (s+1/2)/N}
    nn = np.arange(HOP)
    Winv = np.exp(2j * np.pi * np.outer(s + 0.5, nn) / N)  # [256, 256]
    E = np.concatenate([(2.0 / N) * Winv.real,             # multiplies p
                        -(2.0 / N) * Winv.imag], axis=0)   # multiplies q

    return (np.ascontiguousarray(A, np.float16),
            np.ascontiguousarray(Bm, np.float16),
            np.ascontiguousarray(E, np.float16))


# ---------------------------------------------------------------- bass kernel
def _emit_kernel(ctx, tc, xt_d, amp_d, a_cst, b_cst, e_cst, out_d, n_frames):
    import concourse.mybir as mybir

    nc = tc.nc
    f32 = mybir.dt.float32
    f16 = mybir.dt.float16
    Copy = mybir.ActivationFunctionType.Copy
    mult = mybir.AluOpType.mult
    add = mybir.AluOpType.add
    sub = mybir.AluOpType.subtract

    assert n_frames % BLK == 0
    nfull = n_frames // BLK
    sizes = [BLK] * nfull
    starts = [sum(sizes[:i]) for i in range(len(sizes))]
    nblk = len(sizes)
    PIPE = 2  # inverse trails the forward pass by 2 blocks

    singles = ctx.enter_context(tc.tile_pool(name="singles", bufs=1))
    p_xt = ctx.enter_context(tc.tile_pool(name="p_xt", bufs=3))
    p_at = ctx.enter_context(tc.tile_pool(name="p_at", bufs=3))
    p_h = ctx.enter_context(tc.tile_pool(name="p_h", bufs=3))
    p_x = ctx.enter_context(tc.tile_pool(name="p_x", bufs=3))
    p_t = ctx.enter_context(tc.tile_pool(name="p_t", bufs=2))
    p_m = ctx.enter_context(tc.tile_pool(name="p_m", bufs=PIPE + 2))
    p_o = ctx.enter_context(tc.tile_pool(name="p_o", bufs=3))
    ps_b = ctx.enter_context(tc.tile_pool(name="ps_b", bufs=1, space="PSUM"))
    ps_a = ctx.enter_context(tc.tile_pool(name="ps_a", bufs=2, space="PSUM"))
    ps_o = ctx.enter_context(tc.tile_pool(name="ps_o", bufs=1, space="PSUM"))

    # constants (big ones via SWDGE so the HWDGE queue serves block-0 inputs)
    b_sb = singles.tile([NB, 4, 128], f16)
    nc.sync.dma_start(out=b_sb, in_=b_cst.rearrange("k (c s) -> k c s", s=128))
    a_sb = singles.tile([128, 2, 4, 128], f16)
    nc.gpsimd.dma_start(out=a_sb, in_=a_cst.rearrange(
        "(kt p) (c s) -> p kt c s", p=128, s=128))
    e_sb = singles.tile([128, 4, 2, 128], f16)

    xv = xt_d.rearrange("(kt p) F -> p kt F", p=128)
    av = amp_d
    ov = out_d.rearrange("(jt p) F -> p jt F", p=128)

    # PE warmup: keep PE busy through DMA startup and finish the p-state
    # ramp before real work arrives.
    warm = singles.tile([128, BLK], f16)
    nc.vector.memset(warm, 0.0)
    for w in range(5):
        pw = ps_a.tile([128, 2, BLK], f32, tag="pa")
        nc.tensor.matmul(pw[:, 0, :], warm[:, 0:128], warm,
                         start=True, stop=True)

    m_ring = {}

    for b in range(nblk + PIPE):
        if b < nblk:
            sz = sizes[b]
            lo = starts[b]
            # ---- loads (already fp16, already coeff-major)
            at = p_at.tile([NB, BLK], f16, tag="at")
            nc.sync.dma_start(out=at[:, :sz], in_=av[:, lo:lo + sz])
            xt = p_xt.tile([128, 2, BLK], f16, tag="xt")
            nc.sync.dma_start(out=xt[:, :, :sz], in_=xv[:, :, lo:lo + sz])
            if b == 0:
                nc.gpsimd.dma_start(out=e_sb, in_=e_cst.rearrange(
                    "(c p) (j t) -> p c j t", p=128, t=128))

            # ---- H' = amp @ B, X' = x @ A  (PE), ACT copies to fp16
            h_sb = p_h.tile([128, 4, BLK], f16, tag="h")
            x_sb = p_x.tile([128, 4, BLK], f16, tag="x")
            pb = {}
            pa = {}
            po = mo = None
            o_sb = None
            osz = olo = 0
            if b >= PIPE:
                osz = sizes[b - PIPE]
                olo = starts[b - PIPE]
                mo = m_ring.pop(b - PIPE)
                po_0 = ps_o.tile([128, BLK], f32, tag="po0")
                po_1 = ps_o.tile([128, BLK], f32, tag="po1")
                po = [po_0, po_1]
                o_sb = p_o.tile([128, 2, BLK], f16, tag="o")

            def emit_inv(j, _po=po, _mo=mo, _osz=osz, _olo=olo, _o=o_sb, _b=b):
                for c in range(4):
                    nc.tensor.matmul(_po[j][:, :_osz], e_sb[:, c, j, :],
                                     _mo[:, c, :_osz],
                                     start=(c == 0), stop=(c == 3))
                if _b >= nblk - 1:
                    nc.scalar.activation(out=_o[:, j, :_osz],
                                         in_=_po[j][:, :_osz], func=Copy)
                else:
                    nc.vector.tensor_copy(_o[:, j, :_osz], _po[j][:, :_osz])
                nc.sync.dma_start(out=ov[:, j:j + 1, _olo:_olo + _osz],
                                  in_=_o[:, j:j + 1, :_osz])
            for half in range(2):
                pb_t = ps_b.tile([128, 2, BLK], f32, tag="pb")
                pb[half] = pb_t
                for cc in range(2):
                    c = half * 2 + cc
                    nc.tensor.matmul(pb_t[:, cc, :sz], b_sb[:, c, :],
                                     at[:, :sz], start=True, stop=True)
                pa_t = ps_a.tile([128, 2, BLK], f32, tag="pa")
                pa[half] = pa_t
                for cc in range(2):
                    c = half * 2 + cc
                    for k in range(2):
                        nc.tensor.matmul(pa_t[:, cc, :sz], a_sb[:, k, c, :],
                                         xt[:, k, :sz],
                                         start=(k == 0), stop=(k == 1))
                # copies for this half (ACT), x first (feeds RR)
                nc.scalar.activation(out=x_sb[:, half * 2:half * 2 + 2, :sz],
                                     in_=pa_t[:, :, :sz], func=Copy)
                nc.scalar.activation(out=h_sb[:, half * 2:half * 2 + 2, :sz],
                                     in_=pb_t[:, :, :sz], func=Copy)
                # interleaved inverse t-tile j of block b-PIPE, then its
                # copy + DMA immediately (independent PSUM tile per j)
                if po is not None and b < nblk - 1:
                    emit_inv(half)
                elif b < PIPE:
                    for _w in range(1):
                        pw = ps_a.tile([128, 2, BLK], f32, tag="pa")
                        nc.tensor.matmul(pw[:, 0, :], warm[:, 0:128], warm,
                                         start=True, stop=True)
            if po is not None and b == nblk - 1:
                emit_inv(0)
                emit_inv(1)

            # ---- complex product (DVE fp16 2x): slots [XR|XI] x [HR|HI]
            t_sb = p_t.tile([128, 8, BLK], f16, tag="t")
            m_sb = p_m.tile([128, 4, BLK], f16, tag="m")
            nc.vector.tensor_tensor(out=t_sb[:, 0:2, :sz], in0=x_sb[:, 0:2, :sz],
                                    in1=h_sb[:, 0:2, :sz], op=mult)  # RR
            nc.vector.tensor_tensor(out=t_sb[:, 4:6, :sz], in0=x_sb[:, 0:2, :sz],
                                    in1=h_sb[:, 2:4, :sz], op=mult)  # RI
            nc.vector.tensor_tensor(out=t_sb[:, 6:8, :sz], in0=x_sb[:, 2:4, :sz],
                                    in1=h_sb[:, 0:2, :sz], op=mult)  # IR
            nc.vector.tensor_tensor(out=t_sb[:, 2:4, :sz], in0=x_sb[:, 2:4, :sz],
                                    in1=h_sb[:, 2:4, :sz], op=mult)  # II
            q_eng = nc.vector if b >= nblk - 1 else nc.gpsimd
            q_eng.tensor_tensor(out=m_sb[:, 2:4, :sz],
                                in0=t_sb[:, 4:6, :sz],
                                in1=t_sb[:, 6:8, :sz], op=add)   # q
            nc.vector.tensor_tensor(out=m_sb[:, 0:2, :sz], in0=t_sb[:, 0:2, :sz],
                                    in1=t_sb[:, 2:4, :sz], op=sub)   # p
            m_ring[b] = m_sb

            if b == nblk - 1 and b - 1 >= PIPE - 1 and (b - 1) in m_ring:
                # pull the second-to-last inverse into this block so only one
                # inverse remains in the drain
                osz2 = sizes[b - 1]
                olo2 = starts[b - 1]
                mo2 = m_ring.pop(b - 1)
                o_sb2 = p_o.tile([128, 2, BLK], f16, tag="o")
                for j in range(2):
                    po_j = ps_o.tile([128, BLK], f32, tag=f"po{j}")
                    for c in range(4):
                        nc.tensor.matmul(po_j[:, :osz2], e_sb[:, c, j, :],
                                         mo2[:, c, :osz2],
                                         start=(c == 0), stop=(c == 3))
                    nc.scalar.activation(out=o_sb2[:, j, :osz2],
                                         in_=po_j[:, :osz2], func=Copy)
                    nc.sync.dma_start(out=ov[:, j:j + 1, olo2:olo2 + osz2],
                                      in_=o_sb2[:, j:j + 1, :osz2])
        elif b >= PIPE and (b - PIPE) in m_ring:
            osz = sizes[b - PIPE]
            olo = starts[b - PIPE]
            mo = m_ring.pop(b - PIPE)
            o_sb = p_o.tile([128, 2, BLK], f16, tag="o")
            po_0 = ps_o.tile([128, BLK], f32, tag="po0")
            po_1 = ps_o.tile([128, BLK], f32, tag="po1")
            po_d = [po_0, po_1]
            for j in range(2):
                for c in range(4):
                    nc.tensor.matmul(po_d[j][:, :osz], e_sb[:, c, j, :],
                                     mo[:, c, :osz],
                                     start=(c == 0), stop=(c == 3))
                nc.scalar.activation(out=o_sb[:, j, :osz], in_=po_d[j][:, :osz],
                                     func=Copy)
                nc.sync.dma_start(out=ov[:, j:j + 1, olo:olo + osz],
                                  in_=o_sb[:, j:j + 1, :osz])




def build_nc(n_frames=FR_PER_CORE):
    import concourse.bacc as bacc
    import concourse.mybir as mybir
    import concourse.tile as tile

    f16 = mybir.dt.float16
    f32 = mybir.dt.float32
    nc = bacc.Bacc("TRN2", target_bir_lowering=False, debug=False)
    xt_d = nc.dram_tensor("xt", [HOP, n_frames], f16, kind="ExternalInput").ap()
    amp_d = nc.dram_tensor("ampt", [NB, n_frames], f16, kind="ExternalInput").ap()
    a_cst = nc.dram_tensor("a_cst", [HOP, 512], f16, kind="ExternalInput").ap()
    b_cst = nc.dram_tensor("b_cst", [NB, 512], f16, kind="ExternalInput").ap()
    e_cst = nc.dram_tensor("e_cst", [512, HOP], f16, kind="ExternalInput").ap()
    out_d = nc.dram_tensor("out", [HOP, n_frames], f16, kind="ExternalOutput").ap()

    from contextlib import ExitStack

    with tile.TileContext(nc) as tc, ExitStack() as ctx:
        _emit_kernel(ctx, tc, xt_d, amp_d, a_cst, b_cst, e_cst, out_d, n_frames)
    nc.compile()
    return nc


_CACHE = {}


def _get(n_frames=FR_PER_CORE):
    key = n_frames
    if key not in _CACHE:
        _CACHE[key] = (build_nc(n_frames), _build_matrices())
    return _CACHE[key]


def run_sharded(noise_flat, amp_flat, n_frames_per_core, n_cores, trace=False):
    """noise_flat: [n, 256] fp32 u-noise; amp_flat: [n, 65] fp32."""
    from concourse import bass_utils

    nc, (A, Bm, E) = _get(n_frames_per_core)
    x16 = (2.0 * noise_flat - 1.0).astype(np.float16)
    a16 = amp_flat.astype(np.float16)
    in_maps = []
    for i in range(n_cores):
        lo, hi = i * n_frames_per_core, (i + 1) * n_frames_per_core
        in_maps.append({
            "xt": np.ascontiguousarray(x16[lo:hi].T),
            "ampt": np.ascontiguousarray(a16[lo:hi].T),
            "a_cst": A, "b_cst": Bm, "e_cst": E,
        })
    res = bass_utils.run_bass_kernel_spmd(
        nc, in_maps, core_ids=list(range(n_cores)), trace=trace
    )
    out = np.concatenate(
        [res.results[i]["out"].T for i in range(n_cores)], axis=0)
    return out.astype(np.float32), res


def kernel(filter_bank, noise_u):
    fb = np.asarray(filter_bank, np.float32).reshape(-1, NB)
    nu = np.asarray(noise_u, np.float32).reshape(-1, HOP)
    out, _ = run_sharded(nu, fb, FR_PER_CORE, NCORES)
    return out.reshape(B_DIM, F_DIM * HOP, 1).astype(np.float32)


if __name__ == "__main__":
    nc = build_nc(BLK * 2)
    print("built OK")



# revision 22
# speedup vs baseline: 1.1275x; 1.0014x over previous
"""Trainium2 Bass kernel for nn_NoiseFilter.

Math (negacyclic-transform direct complex product, validated to 2e-14 in f64):
per frame (noise u[256], amp[65]):
    x  = 2u - 1                      (folded into the host-side fp16 cast)
    X' = x @ A        # [512] = (Re | Im) of 256 odd-frequency (negacyclic) bins
    H' = amp @ B      # [512]
    p  = XR*HR - XI*HI               # Re(X'H')  [256]
    q  = XR*HI + XI*HR               # Im(X'H')  [256]
    out = [p|q] @ E   # [256]  negacyclic inverse == linear conv (support 511)

The odd-frequency (negacyclic) DFT has no degenerate real bins: exactly 256
generic complex bins = 512 real slots, so the complex product is 4 bulk
multiplies + 2 bulk add/subs with no special-cased slots.

On-chip dataflow per 512-frame block (inputs host-pre-transposed to
[coeff, frame] layout and pre-cast to fp16, so no on-chip transposes):
    xt   [128,2,512]  <- DMA                 (time-major noise)
    at   [65,512]     <- DMA                 (amp)
    H'   = B-chunks @ at    -> PSUM -> ACT copy   -> h_sb fp16
    X'   = A-chunks @ xt    -> PSUM -> ACT  copy  -> x_sb fp16
    t1..t4, p, q  on DVE (fp16 SBUF, 2x mode)     -> m_sb fp16
    out  = sum_c E[c]^T @ m[c]  -> PSUM -> DVE copy -> DMA (fp16, [t, frame])
Host transposes the [256, frames] fp16 result back and casts fp32.

Steady-state engine budget per 512-frame block: PE 4443 ns (bound, 20
matmuls x 512 rows), ACT 4152 (4 PSUM->SBUF fp16 spectrum copies), DVE
~4290 (4 products @2x + p-combine + out copies), Pool 2127 (q-combine),
DMA ~1.5 us.  The inverse of block n-2 is interleaved into block n's
forward matmuls; warmup matmuls cover the p-state ramp + DMA startup.
Data parallel over 8 cores: 8192 frames/core.
"""

import os

import numpy as np

os.environ.setdefault("MYCRO_LOCAL_CACHE", "1")

HOP = 256
NB = 65
B_DIM = 16
F_DIM = 4096
NCORES = 8
FRAMES = B_DIM * F_DIM
FR_PER_CORE = FRAMES // NCORES  # 8192
BLK = 512                        # frames per block


# ---------------------------------------------------------------- matrices
def _build_matrices():
    FS = 128
    N = 512
    t = np.arange(HOP)
    s = np.arange(256)
    # negacyclic (odd-frequency) analysis: X'[s] = sum_t x[t] e^{-2pi i t (s+1/2)/N}
    W = np.exp(-2j * np.pi * np.outer(t, s + 0.5) / N)   # [256, 256]
    A = np.concatenate([W.real, W.imag], axis=1)          # [256, 512]

    eye = np.eye(NB)
    ir = np.fft.irfft(eye, axis=-1)                       # [65, 128]
    ir = np.roll(ir, FS // 2, axis=-1)
    n = np.arange(FS)
    win = 0.5 * (1.0 - np.cos(2.0 * np.pi * n / FS))
    ir = ir * win
    ir = np.pad(ir, ((0, 0), (0, HOP - FS)))
    M_imp = np.roll(ir, -(FS // 2), axis=-1)              # [65, 256]

    D = M_imp @ W                                         # [65, 256] complex
    Bm = np.concatenate([D.real, D.imag], axis=1)         # [65, 512]

    # inverse: out[n] = (2/N) Re sum_s Y'[s] e^{+2pi i n (s+1/2)/N}
    nn = np.arange(HOP)
    Winv = np.exp(2j * np.pi * np.outer(s + 0.5, nn) / N)  # [256, 256]
    E = np.concatenate([(2.0 / N) * Winv.real,             # multiplies p
                        -(2.0 / N) * Winv.imag], axis=0)   # multiplies q

    return (np.ascontiguousarray(A, np.float16),
            np.ascontiguousarray(Bm, np.float16),
            np.ascontiguousarray(E, np.float16))


# ---------------------------------------------------------------- bass kernel
def _emit_kernel(ctx, tc, xt_d, amp_d, a_cst, b_cst, e_cst, out_d, n_frames):
    import concourse.mybir as mybir

    nc = tc.nc
    f32 = mybir.dt.float32
    f16 = mybir.dt.float16
    Copy = mybir.ActivationFunctionType.Copy
    mult = mybir.AluOpType.mult
    add = mybir.AluOpType.add
    sub = mybir.AluOpType.subtract

    assert n_frames % BLK == 0
    nfull = n_frames // BLK
    sizes = [BLK] * nfull
    starts = [sum(sizes[:i]) for i in range(len(sizes))]
    nblk = len(sizes)
    PIPE = 2  # inverse trails the forward pass by 2 blocks

    singles = ctx.enter_context(tc.tile_pool(name="singles", bufs=1))
    p_xt = ctx.enter_context(tc.tile_pool(name="p_xt", bufs=3))
    p_at = ctx.enter_context(tc.tile_pool(name="p_at", bufs=3))
    p_h = ctx.enter_context(tc.tile_pool(name="p_h", bufs=3))
    p_x = ctx.enter_context(tc.tile_pool(name="p_x", bufs=3))
    p_t = ctx.enter_context(tc.tile_pool(name="p_t", bufs=2))
    p_m = ctx.enter_context(tc.tile_pool(name="p_m", bufs=PIPE + 2))
    p_o = ctx.enter_context(tc.tile_pool(name="p_o", bufs=3))
    ps_b = ctx.enter_context(tc.tile_pool(name="ps_b", bufs=1, space="PSUM"))
    ps_a = ctx.enter_context(tc.tile_pool(name="ps_a", bufs=2, space="PSUM"))
    ps_o = ctx.enter_context(tc.tile_pool(name="ps_o", bufs=1, space="PSUM"))

    # constants (big ones via SWDGE so the HWDGE queue serves block-0 inputs)
    b_sb = singles.tile([NB, 4, 128], f16)
    nc.sync.dma_start(out=b_sb, in_=b_cst.rearrange("k (c s) -> k c s", s=128))
    a_sb = singles.tile([128, 2, 4, 128], f16)
    nc.gpsimd.dma_start(out=a_sb, in_=a_cst.rearrange(
        "(kt p) (c s) -> p kt c s", p=128, s=128))
    e_sb = singles.tile([128, 4, 2, 128], f16)

    xv = xt_d.rearrange("(kt p) F -> p kt F", p=128)
    av = amp_d
    ov = out_d.rearrange("(jt p) F -> p jt F", p=128)

    # PE warmup: keep PE busy through DMA startup and finish the p-state
    # ramp before real work arrives.
    warm = singles.tile([128, BLK], f16)
    nc.vector.memset(warm, 0.0)
    for w in range(5):
        pw = ps_a.tile([128, 2, BLK], f32, tag="pa")
        nc.tensor.matmul(pw[:, 0, :], warm[:, 0:128], warm,
                         start=True, stop=True)

    m_ring = {}

    for b in range(nblk + PIPE):
        if b < nblk:
            sz = sizes[b]
            lo = starts[b]
            # ---- loads (already fp16, already coeff-major)
            at = p_at.tile([NB, BLK], f16, tag="at")
            nc.sync.dma_start(out=at[:, :sz], in_=av[:, lo:lo + sz])
            xt = p_xt.tile([128, 2, BLK], f16, tag="xt")
            nc.sync.dma_start(out=xt[:, :, :sz], in_=xv[:, :, lo:lo + sz])
            if b == 0:
                nc.gpsimd.dma_start(out=e_sb, in_=e_cst.rearrange(
                    "(c p) (j t) -> p c j t", p=128, t=128))

            # ---- H' = amp @ B, X' = x @ A  (PE), ACT copies to fp16
            h_sb = p_h.tile([128, 4, BLK], f16, tag="h")
            x_sb = p_x.tile([128, 4, BLK], f16, tag="x")
            pb = {}
            pa = {}
            po = mo = None
            o_sb = None
            osz = olo = 0
            if b >= PIPE:
                osz = sizes[b - PIPE]
                olo = starts[b - PIPE]
                mo = m_ring.pop(b - PIPE)
                po_0 = ps_o.tile([128, BLK], f32, tag="po0")
                po_1 = ps_o.tile([128, BLK], f32, tag="po1")
                po = [po_0, po_1]
                o_sb = p_o.tile([128, 2, BLK], f16, tag="o")

            def emit_inv(j, _po=po, _mo=mo, _osz=osz, _olo=olo, _o=o_sb, _b=b):
                for c in range(4):
                    nc.tensor.matmul(_po[j][:, :_osz], e_sb[:, c, j, :],
                                     _mo[:, c, :_osz],
                                     start=(c == 0), stop=(c == 3))
                if _b >= nblk - 1:
                    nc.scalar.activation(out=_o[:, j, :_osz],
                                         in_=_po[j][:, :_osz], func=Copy)
                else:
                    nc.vector.tensor_copy(_o[:, j, :_osz], _po[j][:, :_osz])
                nc.sync.dma_start(out=ov[:, j:j + 1, _olo:_olo + _osz],
                                  in_=_o[:, j:j + 1, :_osz])
            for half in range(2):
                pb_t = ps_b.tile([128, 2, BLK], f32, tag="pb")
                pb[half] = pb_t
                for cc in range(2):
                    c = half * 2 + cc
                    nc.tensor.matmul(pb_t[:, cc, :sz], b_sb[:, c, :],
                                     at[:, :sz], start=True, stop=True)
                pa_t = ps_a.tile([128, 2, BLK], f32, tag="pa")
                pa[half] = pa_t
                for cc in range(2):
                    c = half * 2 + cc
                    for k in range(2):
                        nc.tensor.matmul(pa_t[:, cc, :sz], a_sb[:, k, c, :],
                                         xt[:, k, :sz],
                                         start=(k == 0), stop=(k == 1))
                # copies for this half (ACT), x first (feeds RR)
                nc.scalar.activation(out=x_sb[:, half * 2:half * 2 + 2, :sz],
                                     in_=pa_t[:, :, :sz], func=Copy)
                nc.scalar.activation(out=h_sb[:, half * 2:half * 2 + 2, :sz],
                                     in_=pb_t[:, :, :sz], func=Copy)
                # interleaved inverse t-tile j of block b-PIPE, then its
                # copy + DMA immediately (independent PSUM tile per j)
                if po is not None and b < nblk - 1:
                    emit_inv(half)
                elif b < PIPE:
                    for _w in range(1):
                        pw = ps_a.tile([128, 2, BLK], f32, tag="pa")
                        nc.tensor.matmul(pw[:, 0, :], warm[:, 0:128], warm,
                                         start=True, stop=True)
            if po is not None and b == nblk - 1:
                emit_inv(0)
                emit_inv(1)

            # ---- complex product (DVE fp16 2x): slots [XR|XI] x [HR|HI]
            t_sb = p_t.tile([128, 8, BLK], f16, tag="t")
            m_sb = p_m.tile([128, 4, BLK], f16, tag="m")
            nc.vector.tensor_tensor(out=t_sb[:, 0:2, :sz], in0=x_sb[:, 0:2, :sz],
                                    in1=h_sb[:, 0:2, :sz], op=mult)  # RR
            if b >= nblk - 1:  # II early so p (and the inverse's p-chunks) start sooner
                nc.vector.tensor_tensor(out=t_sb[:, 2:4, :sz], in0=x_sb[:, 2:4, :sz],
                                        in1=h_sb[:, 2:4, :sz], op=mult)  # II
                nc.vector.tensor_tensor(out=m_sb[:, 0:2, :sz], in0=t_sb[:, 0:2, :sz],
                                        in1=t_sb[:, 2:4, :sz], op=sub)   # p
            nc.vector.tensor_tensor(out=t_sb[:, 4:6, :sz], in0=x_sb[:, 0:2, :sz],
                                    in1=h_sb[:, 2:4, :sz], op=mult)  # RI
            nc.vector.tensor_tensor(out=t_sb[:, 6:8, :sz], in0=x_sb[:, 2:4, :sz],
                                    in1=h_sb[:, 0:2, :sz], op=mult)  # IR
            if b < nblk - 1:
                nc.vector.tensor_tensor(out=t_sb[:, 2:4, :sz], in0=x_sb[:, 2:4, :sz],
                                        in1=h_sb[:, 2:4, :sz], op=mult)  # II
            q_eng = nc.vector if b >= nblk - 1 else nc.gpsimd
            q_eng.tensor_tensor(out=m_sb[:, 2:4, :sz],
                                in0=t_sb[:, 4:6, :sz],
                                in1=t_sb[:, 6:8, :sz], op=add)   # q
            if b < nblk - 1:
                nc.vector.tensor_tensor(out=m_sb[:, 0:2, :sz],
                                        in0=t_sb[:, 0:2, :sz],
                                        in1=t_sb[:, 2:4, :sz], op=sub)   # p
            m_ring[b] = m_sb

            if b == nblk - 1 and b - 1 >= PIPE - 1 and (b - 1) in m_ring:
                # pull the second-to-last inverse into this block so only one
                # inverse remains in the drain
                osz2 = sizes[b - 1]
                olo2 = starts[b - 1]
                mo2 = m_ring.pop(b - 1)
                o_sb2 = p_o.tile([128, 2, BLK], f16, tag="o")
                for j in range(2):
                    po_j = ps_o.tile([128, BLK], f32, tag=f"po{j}")
                    for c in range(4):
                        nc.tensor.matmul(po_j[:, :osz2], e_sb[:, c, j, :],
                                         mo2[:, c, :osz2],
                                         start=(c == 0), stop=(c == 3))
                    nc.scalar.activation(out=o_sb2[:, j, :osz2],
                                         in_=po_j[:, :osz2], func=Copy)
                    nc.sync.dma_start(out=ov[:, j:j + 1, olo2:olo2 + osz2],
                                      in_=o_sb2[:, j:j + 1, :osz2])
        elif b >= PIPE and (b - PIPE) in m_ring:
            osz = sizes[b - PIPE]
            olo = starts[b - PIPE]
            mo = m_ring.pop(b - PIPE)
            o_sb = p_o.tile([128, 2, BLK], f16, tag="o")
            po_0 = ps_o.tile([128, BLK], f32, tag="po0")
            po_1 = ps_o.tile([128, BLK], f32, tag="po1")
            po_d = [po_0, po_1]
            for j in range(2):
                for c in range(4):
                    nc.tensor.matmul(po_d[j][:, :osz], e_sb[:, c, j, :],
                                     mo[:, c, :osz],
                                     start=(c == 0), stop=(c == 3))
                nc.scalar.activation(out=o_sb[:, j, :osz], in_=po_d[j][:, :osz],
                                     func=Copy)
                nc.sync.dma_start(out=ov[:, j:j + 1, olo:olo + osz],
                                  in_=o_sb[:, j:j + 1, :osz])




def build_nc(n_frames=FR_PER_CORE):
    import concourse.bacc as bacc
    import concourse.mybir as mybir
    import concourse.tile as tile

    f16 = mybir.dt.float16
    f32 = mybir.dt.float32
    nc = bacc.Bacc("TRN2", target_bir_lowering=False, debug=False)
    xt_d = nc.dram_tensor("xt", [HOP, n_frames], f16, kind="ExternalInput").ap()
    amp_d = nc.dram_tensor("ampt", [NB, n_frames], f16, kind="ExternalInput").ap()
    a_cst = nc.dram_tensor("a_cst", [HOP, 512], f16, kind="ExternalInput").ap()
    b_cst = nc.dram_tensor("b_cst", [NB, 512], f16, kind="ExternalInput").ap()
    e_cst = nc.dram_tensor("e_cst", [512, HOP], f16, kind="ExternalInput").ap()
    out_d = nc.dram_tensor("out", [HOP, n_frames], f16, kind="ExternalOutput").ap()

    from contextlib import ExitStack

    with tile.TileContext(nc) as tc, ExitStack() as ctx:
        _emit_kernel(ctx, tc, xt_d, amp_d, a_cst, b_cst, e_cst, out_d, n_frames)
    nc.compile()
    return nc


_CACHE = {}


def _get(n_frames=FR_PER_CORE):
    key = n_frames
    if key not in _CACHE:
        _CACHE[key] = (build_nc(n_frames), _build_matrices())
    return _CACHE[key]


def run_sharded(noise_flat, amp_flat, n_frames_per_core, n_cores, trace=False):
    """noise_flat: [n, 256] fp32 u-noise; amp_flat: [n, 65] fp32."""
    from concourse import bass_utils

    nc, (A, Bm, E) = _get(n_frames_per_core)
    x16 = (2.0 * noise_flat - 1.0).astype(np.float16)
    a16 = amp_flat.astype(np.float16)
    in_maps = []
    for i in range(n_cores):
        lo, hi = i * n_frames_per_core, (i + 1) * n_frames_per_core
        in_maps.append({
            "xt": np.ascontiguousarray(x16[lo:hi].T),
            "ampt": np.ascontiguousarray(a16[lo:hi].T),
            "a_cst": A, "b_cst": Bm, "e_cst": E,
        })
    res = bass_utils.run_bass_kernel_spmd(
        nc, in_maps, core_ids=list(range(n_cores)), trace=trace
    )
    out = np.concatenate(
        [res.results[i]["out"].T for i in range(n_cores)], axis=0)
    return out.astype(np.float32), res


def kernel(filter_bank, noise_u):
    fb = np.asarray(filter_bank, np.float32).reshape(-1, NB)
    nu = np.asarray(noise_u, np.float32).reshape(-1, HOP)
    out, _ = run_sharded(nu, fb, FR_PER_CORE, NCORES)
    return out.reshape(B_DIM, F_DIM * HOP, 1).astype(np.float32)


if __name__ == "__main__":
    nc = build_nc(BLK * 2)
    print("built OK")



# revision 33
# speedup vs baseline: 1.1311x; 1.0032x over previous
"""Trainium2 Bass kernel for nn_NoiseFilter.

Math (negacyclic-transform direct complex product, validated to 2e-14 in f64):
per frame (noise u[256], amp[65]):
    x  = 2u - 1                      (folded into the host-side fp16 cast)
    X' = x @ A        # [512] = (Re | Im) of 256 odd-frequency (negacyclic) bins
    H' = amp @ B      # [512]
    p  = XR*HR - XI*HI               # Re(X'H')  [256]
    q  = XR*HI + XI*HR               # Im(X'H')  [256]
    out = [p|q] @ E   # [256]  negacyclic inverse == linear conv (support 511)

The odd-frequency (negacyclic) DFT has no degenerate real bins: exactly 256
generic complex bins = 512 real slots, so the complex product is 4 bulk
multiplies + 2 bulk add/subs with no special-cased slots.

On-chip dataflow per 512-frame block (inputs host-pre-transposed to
[coeff, frame] layout and pre-cast to fp16, so no on-chip transposes):
    xt   [128,2,512]  <- DMA                 (time-major noise)
    at   [65,512]     <- DMA                 (amp)
    H'   = B-chunks @ at    -> PSUM -> ACT copy   -> h_sb fp16
    X'   = A-chunks @ xt    -> PSUM -> ACT  copy  -> x_sb fp16
    t1..t4, p, q  on DVE (fp16 SBUF, 2x mode)     -> m_sb fp16
    out  = sum_c E[c]^T @ m[c]  -> PSUM -> DVE copy -> DMA (fp16, [t, frame])
Host transposes the [256, frames] fp16 result back and casts fp32.

Steady-state engine budget per 512-frame block: PE 4443 ns (bound, 20
matmuls x 512 rows), ACT 4152 (4 PSUM->SBUF fp16 spectrum copies), DVE
~4290 (4 products @2x + p-combine + out copies), Pool 2127 (q-combine),
DMA ~1.5 us.  The inverse of block n-2 is interleaved into block n's
forward matmuls; warmup matmuls cover the p-state ramp + DMA startup.
Data parallel over 8 cores: 8192 frames/core.
"""

import os

import numpy as np

os.environ.setdefault("MYCRO_LOCAL_CACHE", "1")

HOP = 256
NB = 65
B_DIM = 16
F_DIM = 4096
NCORES = 8
FRAMES = B_DIM * F_DIM
FR_PER_CORE = FRAMES // NCORES  # 8192
BLK = 512                        # frames per block


# ---------------------------------------------------------------- matrices
def _build_matrices():
    FS = 128
    N = 512
    t = np.arange(HOP)
    s = np.arange(256)
    # negacyclic (odd-frequency) analysis: X'[s] = sum_t x[t] e^{-2pi i t (s+1/2)/N}
    W = np.exp(-2j * np.pi * np.outer(t, s + 0.5) / N)   # [256, 256]
    A = np.concatenate([W.real, W.imag], axis=1)          # [256, 512]

    eye = np.eye(NB)
    ir = np.fft.irfft(eye, axis=-1)                       # [65, 128]
    ir = np.roll(ir, FS // 2, axis=-1)
    n = np.arange(FS)
    win = 0.5 * (1.0 - np.cos(2.0 * np.pi * n / FS))
    ir = ir * win
    ir = np.pad(ir, ((0, 0), (0, HOP - FS)))
    M_imp = np.roll(ir, -(FS // 2), axis=-1)              # [65, 256]

    D = M_imp @ W                                         # [65, 256] complex
    Bm = np.concatenate([D.real, D.imag], axis=1)         # [65, 512]

    # inverse: out[n] = (2/N) Re sum_s Y'[s] e^{+2pi i n (s+1/2)/N}
    nn = np.arange(HOP)
    Winv = np.exp(2j * np.pi * np.outer(s + 0.5, nn) / N)  # [256, 256]
    E = np.concatenate([(2.0 / N) * Winv.real,             # multiplies p
                        -(2.0 / N) * Winv.imag], axis=0)   # multiplies q

    return (np.ascontiguousarray(A, np.float16),
            np.ascontiguousarray(Bm, np.float16),
            np.ascontiguousarray(E, np.float16))


# ---------------------------------------------------------------- bass kernel
def _emit_kernel(ctx, tc, xt_d, amp_d, a_cst, b_cst, e_cst, out_d, n_frames):
    import concourse.mybir as mybir

    nc = tc.nc
    f32 = mybir.dt.float32
    f16 = mybir.dt.float16
    Copy = mybir.ActivationFunctionType.Copy
    mult = mybir.AluOpType.mult
    add = mybir.AluOpType.add
    sub = mybir.AluOpType.subtract

    assert n_frames % BLK == 0
    nfull = n_frames // BLK
    sizes = [BLK] * nfull
    starts = [sum(sizes[:i]) for i in range(len(sizes))]
    nblk = len(sizes)
    PIPE = 2  # inverse trails the forward pass by 2 blocks

    singles = ctx.enter_context(tc.tile_pool(name="singles", bufs=1))
    p_xt = ctx.enter_context(tc.tile_pool(name="p_xt", bufs=3))
    p_at = ctx.enter_context(tc.tile_pool(name="p_at", bufs=3))
    p_h = ctx.enter_context(tc.tile_pool(name="p_h", bufs=3))
    p_x = ctx.enter_context(tc.tile_pool(name="p_x", bufs=3))
    p_t = ctx.enter_context(tc.tile_pool(name="p_t", bufs=2))
    p_m = ctx.enter_context(tc.tile_pool(name="p_m", bufs=PIPE + 2))
    p_o = ctx.enter_context(tc.tile_pool(name="p_o", bufs=3))
    ps_b = ctx.enter_context(tc.tile_pool(name="ps_b", bufs=1, space="PSUM"))
    ps_a = ctx.enter_context(tc.tile_pool(name="ps_a", bufs=2, space="PSUM"))
    ps_o = ctx.enter_context(tc.tile_pool(name="ps_o", bufs=1, space="PSUM"))

    # constants (big ones via SWDGE so the HWDGE queue serves block-0 inputs)
    b_sb = singles.tile([NB, 4, 128], f16)
    nc.sync.dma_start(out=b_sb, in_=b_cst.rearrange("k (c s) -> k c s", s=128))
    a_sb = singles.tile([128, 2, 4, 128], f16)
    nc.gpsimd.dma_start(out=a_sb, in_=a_cst.rearrange(
        "(kt p) (c s) -> p kt c s", p=128, s=128))
    e_sb = singles.tile([128, 4, 2, 128], f16)

    xv = xt_d.rearrange("(kt p) F -> p kt F", p=128)
    av = amp_d
    ov = out_d.rearrange("(jt p) F -> p jt F", p=128)

    # PE warmup: keep PE busy through DMA startup and finish the p-state
    # ramp before real work arrives.
    warm = singles.tile([128, BLK], f16)
    nc.vector.memset(warm, 0.0)
    for w in range(7):
        pw = ps_a.tile([128, 2, BLK], f32, tag="pa")
        nc.tensor.matmul(pw[:, 0, :], warm[:, 0:128], warm,
                         start=True, stop=True)

    m_ring = {}

    for b in range(nblk + PIPE):
        if b < nblk:
            sz = sizes[b]
            lo = starts[b]
            # ---- loads (already fp16, already coeff-major)
            at = p_at.tile([NB, BLK], f16, tag="at")
            nc.sync.dma_start(out=at[:, :sz], in_=av[:, lo:lo + sz])
            xt = p_xt.tile([128, 2, BLK], f16, tag="xt")
            nc.sync.dma_start(out=xt[:, :, :sz], in_=xv[:, :, lo:lo + sz])
            if b == 0:
                nc.gpsimd.dma_start(out=e_sb, in_=e_cst.rearrange(
                    "(c p) (j t) -> p c j t", p=128, t=128))

            # ---- H' = amp @ B, X' = x @ A  (PE), ACT copies to fp16
            h_sb = p_h.tile([128, 4, BLK], f16, tag="h")
            x_sb = p_x.tile([128, 4, BLK], f16, tag="x")
            pb = {}
            pa = {}
            po = mo = None
            o_sb = None
            osz = olo = 0
            if b >= PIPE:
                osz = sizes[b - PIPE]
                olo = starts[b - PIPE]
                mo = m_ring.pop(b - PIPE)
                po_0 = ps_o.tile([128, BLK], f32, tag="po0")
                po_1 = ps_o.tile([128, BLK], f32, tag="po1")
                po = [po_0, po_1]
                o_sb = p_o.tile([128, 2, BLK], f16, tag="o")

            def emit_inv(j, _po=po, _mo=mo, _osz=osz, _olo=olo, _o=o_sb, _b=b):
                for c in range(4):
                    nc.tensor.matmul(_po[j][:, :_osz], e_sb[:, c, j, :],
                                     _mo[:, c, :_osz],
                                     start=(c == 0), stop=(c == 3))
                if _b >= nblk - 1:
                    nc.scalar.activation(out=_o[:, j, :_osz],
                                         in_=_po[j][:, :_osz], func=Copy)
                else:
                    nc.vector.tensor_copy(_o[:, j, :_osz], _po[j][:, :_osz])
                nc.sync.dma_start(out=ov[:, j:j + 1, _olo:_olo + _osz],
                                  in_=_o[:, j:j + 1, :_osz])
            for half in range(2):
                pb_t = ps_b.tile([128, 2, BLK], f32, tag="pb")
                pb[half] = pb_t
                for cc in range(2):
                    c = half * 2 + cc
                    nc.tensor.matmul(pb_t[:, cc, :sz], b_sb[:, c, :],
                                     at[:, :sz], start=True, stop=True)
                pa_t = ps_a.tile([128, 2, BLK], f32, tag="pa")
                pa[half] = pa_t
                for cc in range(2):
                    c = half * 2 + cc
                    for k in range(2):
                        nc.tensor.matmul(pa_t[:, cc, :sz], a_sb[:, k, c, :],
                                         xt[:, k, :sz],
                                         start=(k == 0), stop=(k == 1))
                # copies for this half (ACT), x first (feeds RR)
                nc.scalar.activation(out=x_sb[:, half * 2:half * 2 + 2, :sz],
                                     in_=pa_t[:, :, :sz], func=Copy)
                nc.scalar.activation(out=h_sb[:, half * 2:half * 2 + 2, :sz],
                                     in_=pb_t[:, :, :sz], func=Copy)
                # interleaved inverse t-tile j of block b-PIPE, then its
                # copy + DMA immediately (independent PSUM tile per j)
                if po is not None and b < nblk - 1:
                    emit_inv(half)
                elif b < PIPE:
                    for _w in range(1):
                        pw = ps_a.tile([128, 2, BLK], f32, tag="pa")
                        nc.tensor.matmul(pw[:, 0, :], warm[:, 0:128], warm,
                                         start=True, stop=True)
            if po is not None and b == nblk - 1:
                emit_inv(0)
                emit_inv(1)

            # ---- complex product (DVE fp16 2x): slots [XR|XI] x [HR|HI]
            t_sb = p_t.tile([128, 8, BLK], f16, tag="t")
            m_sb = p_m.tile([128, 4, BLK], f16, tag="m")
            nc.vector.tensor_tensor(out=t_sb[:, 0:2, :sz], in0=x_sb[:, 0:2, :sz],
                                    in1=h_sb[:, 0:2, :sz], op=mult)  # RR
            if b >= nblk - 1:  # II early so p (and the inverse's p-chunks) start sooner
                nc.vector.tensor_tensor(out=t_sb[:, 2:4, :sz], in0=x_sb[:, 2:4, :sz],
                                        in1=h_sb[:, 2:4, :sz], op=mult)  # II
                nc.vector.tensor_tensor(out=m_sb[:, 0:2, :sz], in0=t_sb[:, 0:2, :sz],
                                        in1=t_sb[:, 2:4, :sz], op=sub)   # p
            nc.vector.tensor_tensor(out=t_sb[:, 4:6, :sz], in0=x_sb[:, 0:2, :sz],
                                    in1=h_sb[:, 2:4, :sz], op=mult)  # RI
            nc.vector.tensor_tensor(out=t_sb[:, 6:8, :sz], in0=x_sb[:, 2:4, :sz],
                                    in1=h_sb[:, 0:2, :sz], op=mult)  # IR
            if b < nblk - 1:
                nc.vector.tensor_tensor(out=t_sb[:, 2:4, :sz], in0=x_sb[:, 2:4, :sz],
                                        in1=h_sb[:, 2:4, :sz], op=mult)  # II
            q_eng = nc.vector if b >= nblk - 1 else nc.gpsimd
            q_eng.tensor_tensor(out=m_sb[:, 2:4, :sz],
                                in0=t_sb[:, 4:6, :sz],
                                in1=t_sb[:, 6:8, :sz], op=add)   # q
            if b < nblk - 1:
                nc.vector.tensor_tensor(out=m_sb[:, 0:2, :sz],
                                        in0=t_sb[:, 0:2, :sz],
                                        in1=t_sb[:, 2:4, :sz], op=sub)   # p
            m_ring[b] = m_sb

            if b == nblk - 1 and b - 1 >= PIPE - 1 and (b - 1) in m_ring:
                # pull the second-to-last inverse into this block so only one
                # inverse remains in the drain
                osz2 = sizes[b - 1]
                olo2 = starts[b - 1]
                mo2 = m_ring.pop(b - 1)
                o_sb2 = p_o.tile([128, 2, BLK], f16, tag="o")
                for j in range(2):
                    po_j = ps_o.tile([128, BLK], f32, tag=f"po{j}")
                    for c in range(4):
                        nc.tensor.matmul(po_j[:, :osz2], e_sb[:, c, j, :],
                                         mo2[:, c, :osz2],
                                         start=(c == 0), stop=(c == 3))
                    nc.scalar.activation(out=o_sb2[:, j, :osz2],
                                         in_=po_j[:, :osz2], func=Copy)
                    nc.sync.dma_start(out=ov[:, j:j + 1, olo2:olo2 + osz2],
                                      in_=o_sb2[:, j:j + 1, :osz2])
        elif b >= PIPE and (b - PIPE) in m_ring:
            osz = sizes[b - PIPE]
            olo = starts[b - PIPE]
            mo = m_ring.pop(b - PIPE)
            o_sb = p_o.tile([128, 2, BLK], f16, tag="o")
            po_0 = ps_o.tile([128, BLK], f32, tag="po0")
            po_1 = ps_o.tile([128, BLK], f32, tag="po1")
            po_d = [po_0, po_1]
            for j in range(2):
                for c in range(4):
                    nc.tensor.matmul(po_d[j][:, :osz], e_sb[:, c, j, :],
                                     mo[:, c, :osz],
                                     start=(c == 0), stop=(c == 3))
                nc.scalar.activation(out=o_sb[:, j, :osz], in_=po_d[j][:, :osz],
                                     func=Copy)
                # j0 via SWDGE so the final j1 HWDGE gen isn't queued behind it
                eng = nc.gpsimd if j == 0 else nc.sync
                eng.dma_start(out=ov[:, j:j + 1, olo:olo + osz],
                              in_=o_sb[:, j:j + 1, :osz])




def build_nc(n_frames=FR_PER_CORE):
    import concourse.bacc as bacc
    import concourse.mybir as mybir
    import concourse.tile as tile

    f16 = mybir.dt.float16
    f32 = mybir.dt.float32
    nc = bacc.Bacc("TRN2", target_bir_lowering=False, debug=False)
    xt_d = nc.dram_tensor("xt", [HOP, n_frames], f16, kind="ExternalInput").ap()
    amp_d = nc.dram_tensor("ampt", [NB, n_frames], f16, kind="ExternalInput").ap()
    a_cst = nc.dram_tensor("a_cst", [HOP, 512], f16, kind="ExternalInput").ap()
    b_cst = nc.dram_tensor("b_cst", [NB, 512], f16, kind="ExternalInput").ap()
    e_cst = nc.dram_tensor("e_cst", [512, HOP], f16, kind="ExternalInput").ap()
    out_d = nc.dram_tensor("out", [HOP, n_frames], f16, kind="ExternalOutput").ap()

    from contextlib import ExitStack

    with tile.TileContext(nc) as tc, ExitStack() as ctx:
        _emit_kernel(ctx, tc, xt_d, amp_d, a_cst, b_cst, e_cst, out_d, n_frames)
    nc.compile()
    return nc


_CACHE = {}


def _get(n_frames=FR_PER_CORE):
    key = n_frames
    if key not in _CACHE:
        _CACHE[key] = (build_nc(n_frames), _build_matrices())
    return _CACHE[key]


def run_sharded(noise_flat, amp_flat, n_frames_per_core, n_cores, trace=False):
    """noise_flat: [n, 256] fp32 u-noise; amp_flat: [n, 65] fp32."""
    from concourse import bass_utils

    nc, (A, Bm, E) = _get(n_frames_per_core)
    x16 = (2.0 * noise_flat - 1.0).astype(np.float16)
    a16 = amp_flat.astype(np.float16)
    in_maps = []
    for i in range(n_cores):
        lo, hi = i * n_frames_per_core, (i + 1) * n_frames_per_core
        in_maps.append({
            "xt": np.ascontiguousarray(x16[lo:hi].T),
            "ampt": np.ascontiguousarray(a16[lo:hi].T),
            "a_cst": A, "b_cst": Bm, "e_cst": E,
        })
    res = bass_utils.run_bass_kernel_spmd(
        nc, in_maps, core_ids=list(range(n_cores)), trace=trace
    )
    out = np.concatenate(
        [res.results[i]["out"].T for i in range(n_cores)], axis=0)
    return out.astype(np.float32), res


def kernel(filter_bank, noise_u):
    fb = np.asarray(filter_bank, np.float32).reshape(-1, NB)
    nu = np.asarray(noise_u, np.float32).reshape(-1, HOP)
    out, _ = run_sharded(nu, fb, FR_PER_CORE, NCORES)
    return out.reshape(B_DIM, F_DIM * HOP, 1).astype(np.float32)


if __name__ == "__main__":
    nc = build_nc(BLK * 2)
    print("built OK")

